# revision 33
# baseline (speedup 1.0000x reference)
"""Trainium2 Bass kernel for the GSAT HeteroGNN problem (8 NeuronCores).

Self-contained: hardcodes shapes/sharding; only imports the concourse
toolchain.

Strategy (dst-node sharding, SPMD over 8 cores):
  - papers split into 8 canonical chunks of 12500 (padded 12544 = 98 tiles),
    authors 8 x 6250 (padded 6272 = 49 tiles).
  - edges live on their dst's owner core, laid out host-side into 128-slot
    columns per (4-tile window, src-bank); dma_gather (int16 idx) fetches
    fp8 source rows as [128, cols, feat].
  - segment-mean via host-precomputed fp8 masks streamed by DMA:
    mask[slot, dst_in_window] = 1/deg(dst); TensorE accumulates
    aggT[feat, 512] in PSUM per window (no on-device mask building).
  - L1 gathers read per-core COMPACT fp8 tables (only the <=32k rows this
    core references -> single bank, minimal column padding).
  - L1 outputs h1 are written twice: fp8 rows into a local chunk that a
    Shared-output AllGather assembles into a shared fp8 table (each rank
    contributes only its 1.6-3.2MB shard; the old Local-output AllGathers
    moved 77MB/core), and fp16 into a local chunk used for DMA transposes
    (L2 root terms).
  - L2 gathers read the shared fp8 h1 tables directly.
  - all DMA transposes are placed before any collective in program order
    (the scheduler serializes transposes with collectives).
  - global mean-pool via ones-column matmuls accumulating in PSUM; final
    2-layer MLP on host in fp64.
"""
import os
import sys

try:
    import concourse  # noqa: F401
except ImportError:  # toolchain location in the grading container
    sys.path.insert(0, "/opt/trn_rl_repo")

import numpy as np
import ml_dtypes
from concourse import bass, bacc, mybir, tile  # noqa: F401
from concourse import bass_utils
from concourse.bass import _add_dep_helper

dt = mybir.dt
F8 = ml_dtypes.float8_e4m3

# ---------------------------------------------------------------- constants
NA, NP_, E = 50000, 100000, 300000
IN, H, OUT = 128, 256, 16
C = 8                      # cores
P = 128                    # partitions
A_CAN, P_CAN = NA // C, NP_ // C              # 6250 / 12500
A_PAD = ((A_CAN + P - 1) // P) * P            # 6272
P_PAD = ((P_CAN + P - 1) // P) * P            # 12544
NA_AG, NP_AG = C * A_PAD, C * P_PAD           # 50176 / 100352
WIN = 4                    # dst tiles per PSUM window (512 dsts)
WD = WIN * P               # window width in dsts


class RelLayer:
    """Host-side layout for one (relation, layer): slot columns per
    (window, bank), uniform across cores (max-over-cores column counts),
    int16 gather indices and fp8 recip masks."""

    def __init__(self, row_of, dst_owner, dstl, n_dst_can, n_dst_pad,
                 recip_dst_local, table_rows):
        # row_of: [C] list of per-edge row ids (into this layer's table)
        # dst_owner/dstl: per-edge owner core and local dst id (global arrays
        # already split: row_of[c] aligned with dstl[c])
        self.n_tiles = n_dst_pad // P
        self.n_win = (self.n_tiles + WIN - 1) // WIN
        nb = (table_rows + 32767) // 32768
        self.n_banks = nb
        self.bank_rows = (table_rows + nb - 1) // nb
        self.table_rows = table_rows

        # per-core per-cell counts -> uniform ncols
        ncols = np.zeros((self.n_win, nb), np.int64)
        per_core = []
        for c in range(C):
            rows, dl = row_of[c], dstl[c]
            w = dl // WD
            b = rows // self.bank_rows
            cnt = np.zeros((self.n_win, nb), np.int64)
            np.add.at(cnt, (w, b), 1)
            ncols = np.maximum(ncols, (cnt + P - 1) // P)
            per_core.append((rows, dl, w, b))
        self.ncols = ncols

        # global column layout: window-major, bank-minor
        self.col_base = np.zeros(self.n_win + 1, np.int64)
        self.ops = []              # per window: list of (bank, ioff, nidx, lcb)
        ioff = 0
        col = 0
        for w in range(self.n_win):
            self.col_base[w] = col
            wops = []
            lcb = 0
            for b in range(nb):
                nco = int(ncols[w, b])
                if nco:
                    wops.append((b, ioff, nco * P, lcb))
                    ioff += nco * P // 16
                    lcb += nco
                    col += nco
            self.ops.append(wops)
        self.col_base[self.n_win] = col
        self.total_cols = col
        self.idx_width = ioff
        self.wcols = np.diff(self.col_base).astype(np.int64)
        self.max_wcols = int(self.wcols.max()) if col else 0
        self.total_idx = col * P

        # per-core idx + masks
        self.idx16 = np.zeros((C, P, max(self.idx_width, 1)), np.int16)
        self.masks = np.zeros((C, P, max(col, 1), WD), F8)
        cell_base = {}
        lcb_map = {}
        for w in range(self.n_win):
            for (b, io, nidx, lcb) in self.ops[w]:
                cell_base[(w, b)] = io
                lcb_map[(w, b)] = self.col_base[w] + lcb
        for c in range(C):
            rows, dl, w_e, b_e = per_core[c]
            order = np.argsort(w_e * nb + b_e, kind="stable")
            rows_s, dl_s, w_s, b_s = rows[order], dl[order], w_e[order], b_e[order]
            rec_s = recip_dst_local[c][dl_s].astype(np.float32)
            rib_s = (rows_s % self.bank_rows).astype(np.int64)
            # rank within each (w, b) run
            key = w_s * nb + b_s
            # j = index within cell
            cellcnt = np.bincount(key, minlength=self.n_win * nb)
            starts = np.zeros(self.n_win * nb + 1, np.int64)
            np.cumsum(cellcnt, out=starts[1:])
            j = np.arange(len(key)) - starts[key]
            # idx slab (flat over ops)
            flat = np.zeros(max(self.idx_width, 1) * 16, np.int16)
            iobase = np.array([cell_base.get((w, b), -1) * 16
                               for w in range(self.n_win) for b in range(nb)]
                              ).reshape(self.n_win, nb)
            pos = iobase[w_s, b_s] + j
            flat[pos] = rib_s.astype(np.int16)
            w16 = flat.reshape(-1, 16).T       # [16, width]
            self.idx16[c] = np.tile(w16, (8, 1))
            # masks
            gcol = np.array([lcb_map.get((w, b), 0)
                             for w in range(self.n_win) for b in range(nb)]
                            ).reshape(self.n_win, nb)
            cc = gcol[w_s, b_s] + j // P
            pp = j % P
            off = dl_s - w_s * WD
            self.masks[c][pp, cc, off] = rec_s.astype(F8)


def _balance_perm(deg, n_nodes, can):
    """Permutation node -> new global id, dealing nodes into (core, window)
    cells so per-cell degree sums are balanced (pool is perm-invariant)."""
    import heapq
    n_win = ((can + P - 1) // P + WIN - 1) // WIN
    caps, base = [], []
    for c in range(C):
        for w in range(n_win):
            cap = min(WD, can - w * WD)
            caps.append(cap)
            base.append(c * can + w * WD)
    order = np.argsort(-deg, kind="stable")
    heap = [(0.0, i) for i in range(len(caps))]
    heapq.heapify(heap)
    fill = np.zeros(len(caps), np.int64)
    perm = np.empty(n_nodes, np.int64)
    for nd in order:
        while True:
            s, i = heapq.heappop(heap)
            if fill[i] < caps[i]:
                break
        perm[nd] = base[i] + fill[i]
        fill[i] += 1
        if fill[i] < caps[i]:
            heapq.heappush(heap, (s + deg[nd], i))
    return perm


def _prep(inputs):
    f = lambda k: np.asarray(inputs[k], np.float32)
    x_author, x_paper = f("x_author"), f("x_paper")
    ws, wd = (np.asarray(inputs["ei_writes_src"], np.int64),
              np.asarray(inputs["ei_writes_dst"], np.int64))
    bs, bd = (np.asarray(inputs["ei_wb_src"], np.int64),
              np.asarray(inputs["ei_wb_dst"], np.int64))

    # relabel nodes so per-(core, window) dst-degree sums are balanced
    pa_perm = _balance_perm(np.bincount(wd, minlength=NP_), NP_, P_CAN)
    au_perm = _balance_perm(np.bincount(bd, minlength=NA), NA, A_CAN)
    inv_pa = np.empty(NP_, np.int64)
    inv_pa[pa_perm] = np.arange(NP_)
    inv_au = np.empty(NA, np.int64)
    inv_au[au_perm] = np.arange(NA)
    x_paper = x_paper[inv_pa]
    x_author = x_author[inv_au]
    wd, bs = pa_perm[wd], pa_perm[bs]
    ws, bd = au_perm[ws], au_perm[bd]

    cnt_p = np.bincount(wd, minlength=NP_).astype(np.float32)
    cnt_a = np.bincount(bd, minlength=NA).astype(np.float32)
    recip_p = 1.0 / np.maximum(cnt_p, 1.0)
    recip_a = 1.0 / np.maximum(cnt_a, 1.0)

    # split edges by dst owner
    def split(src, dst, dst_can):
        srcs, dstls = [], []
        for c in range(C):
            m = (dst // dst_can) == c
            srcs.append(src[m])
            dstls.append((dst[m] % dst_can).astype(np.int64))
        return srcs, dstls

    w_src, w_dstl = split(ws, wd, P_CAN)     # writes: dst papers
    b_src, b_dstl = split(bs, bd, A_CAN)     # wb: dst authors

    # L1 compact tables (per-core unique srcs)
    uniqW = [np.unique(s) for s in w_src]    # authors referenced per core
    uniqB = [np.unique(s) for s in b_src]    # papers referenced per core
    rowsW = ((max(len(u) for u in uniqW) + P - 1) // P) * P
    rowsB = ((max(len(u) for u in uniqB) + P - 1) // P) * P
    assert rowsW <= 32768 and rowsB <= 32768
    xa_cmp = np.zeros((C, rowsW, IN), np.float16)
    xp_cmp = np.zeros((C, rowsB, IN), np.float16)
    for c in range(C):
        xa_cmp[c, :len(uniqW[c])] = x_author[uniqW[c]].astype(np.float16)
        xp_cmp[c, :len(uniqB[c])] = x_paper[uniqB[c]].astype(np.float16)

    recip_p_loc = [recip_p[c * P_CAN:(c + 1) * P_CAN] for c in range(C)]
    recip_a_loc = [recip_a[c * A_CAN:(c + 1) * A_CAN] for c in range(C)]

    # AG row mapping for L2 tables
    agW = [(s // A_CAN) * A_PAD + (s % A_CAN) for s in w_src]
    agB = [(s // P_CAN) * P_PAD + (s % P_CAN) for s in b_src]
    cmpW = [np.searchsorted(uniqW[c], w_src[c]) for c in range(C)]
    cmpB = [np.searchsorted(uniqB[c], b_src[c]) for c in range(C)]

    rels = dict(
        W1=RelLayer(cmpW, None, w_dstl, P_CAN, P_PAD, recip_p_loc, rowsW),
        B1=RelLayer(cmpB, None, b_dstl, A_CAN, A_PAD, recip_a_loc, rowsB),
        W2=RelLayer(agW, None, w_dstl, P_CAN, P_PAD, recip_p_loc, NA_AG),
        B2=RelLayer(agB, None, b_dstl, A_CAN, A_PAD, recip_a_loc, NP_AG),
    )

    # fp16 local chunks (root/skip transposes)
    xa_chunk = np.zeros((C, A_PAD, IN), np.float16)
    xp_chunk = np.zeros((C, P_PAD, IN), np.float16)
    for c in range(C):
        xa_chunk[c, :A_CAN] = x_author[c * A_CAN:(c + 1) * A_CAN]
        xp_chunk[c, :P_CAN] = x_paper[c * P_CAN:(c + 1) * P_CAN]

    # weight slab: 14 x [128, 256] fp16 (transposed: [in, out])
    wT = lambda k: f(k).T.astype(np.float16)
    slabs = [wT("c1w_Wl"), wT("c1w_Wr"), wT("c1b_Wl"), wT("c1b_Wr")]
    for k in ("c2w_Wl", "c2w_Wr", "c2b_Wl", "c2b_Wr"):
        w2 = wT(k)
        slabs += [w2[:128], w2[128:]]
    slabs += [wT("skipA_W"), wT("skipP_W")]
    wslab = np.concatenate(slabs, axis=0)          # [14*128, 256]

    pool_ones = np.zeros((P, 3), np.float16)
    pool_ones[:, 0] = 1.0
    pool_ones[:P_CAN - (P_PAD // P - 1) * P, 1] = 1.0   # last paper tile mask
    pool_ones[:A_CAN - (A_PAD // P - 1) * P, 2] = 1.0   # last author tile mask

    bias_nz = {k: bool(np.any(f(k))) for k in
               ("c1w_bl", "c1b_bl", "skipA_b", "skipP_b")}
    bias_arr = {k: np.broadcast_to(f(k2), (P, H)).astype(np.float32).copy()
                for k, k2 in (("bias_p1", "c1w_bl"), ("bias_a1", "c1b_bl"),
                              ("bias_p2", "skipP_b"), ("bias_a2", "skipA_b"))}

    in_maps = []
    for c in range(C):
        m = dict(
            xa_cmp=xa_cmp[c], xp_cmp=xp_cmp[c],
            xa_chunk=xa_chunk[c], xp_chunk=xp_chunk[c],
            wslab=wslab, pool_ones=pool_ones,
        )
        for nm, rl in rels.items():
            m["idx_" + nm] = rl.idx16[c]
            m["mask_" + nm] = rl.masks[c].reshape(P, -1)
        for k, arr in bias_arr.items():
            m[k] = arr
        in_maps.append(m)
    return rels, in_maps, bias_nz


def _build(rels, bias_nz, debug=False):
    nc = bacc.Bacc("TRN2", target_bir_lowering=False, debug=False,
                   num_devices=C)
    f16, f32, i16, f8 = dt.float16, dt.float32, dt.int16, dt.float8e4
    ein = lambda n, s, d: nc.dram_tensor(n, s, d, kind="ExternalInput")

    xa_cmp = ein("xa_cmp", [rels["W1"].table_rows, IN], f16)
    xp_cmp = ein("xp_cmp", [rels["B1"].table_rows, IN], f16)
    xa_chunk = ein("xa_chunk", [A_PAD, IN], f16)
    xp_chunk = ein("xp_chunk", [P_PAD, IN], f16)
    wslab = ein("wslab", [14 * P, H], f16)
    pool_in = ein("pool_ones", [P, 3], f16)
    idx_h, mask_h = {}, {}
    for nm, rl in rels.items():
        idx_h[nm] = ein("idx_" + nm, [P, max(rl.idx_width, 1)], i16)
        mask_h[nm] = ein("mask_" + nm, [P, max(rl.total_cols, 1) * WD], f8)
    bias_in = {k: ein(k, [P, H], f32)
               for k in ("bias_p1", "bias_a1", "bias_p2", "bias_a2")}

    out_pool = nc.dram_tensor("out_pool", [1, 2 * H], f32,
                              kind="ExternalOutput")
    if debug:
        dbg_h1a = nc.dram_tensor("dbg_h1a", [A_PAD, H], f16,
                                 kind="ExternalOutput")
        dbg_h1p = nc.dram_tensor("dbg_h1p", [P_PAD, H], f16,
                                 kind="ExternalOutput")

    W = {k: i for i, k in enumerate(
        ["c1w_Wl", "c1w_Wr", "c1b_Wl", "c1b_Wr",
         "c2w_Wl0", "c2w_Wl1", "c2w_Wr0", "c2w_Wr1",
         "c2b_Wl0", "c2b_Wl1", "c2b_Wr0", "c2b_Wr1",
         "skipA_W", "skipP_W"])}
    relu_f = mybir.ActivationFunctionType.Relu
    rg = [list(range(C))]
    MAXW = max(rl.max_wcols for rl in rels.values())

    with tile.TileContext(nc) as tc:
        with tc.tile_pool(name="persist", bufs=1) as pp, \
             tc.tile_pool(name="dram", bufs=1, space="DRAM") as dp, \
             tc.tile_pool(name="work", bufs=3) as wk, \
             tc.tile_pool(name="msgs", bufs=2) as mp, \
             tc.tile_pool(name="maskp", bufs=2) as mk, \
             tc.tile_pool(name="psA", bufs=4, space="PSUM") as psA, \
             tc.tile_pool(name="psL", bufs=2, space="PSUM") as psL, \
             tc.tile_pool(name="psP", bufs=1, space="PSUM") as psP:

            # ---------------- persistent loads (idx first: gathers need it)
            idx_t = {}
            for nm in ("B1", "W1", "W2", "B2"):
                rl = rels[nm]
                t = pp.tile([P, max(rl.idx_width, 1)], i16, name="idx" + nm)
                nc.sync.dma_start(out=t[:], in_=idx_h[nm][:])
                idx_t[nm] = t
            wt = pp.tile([P, 14, H], f16, name="wt", tag="wt")
            nc.sync.dma_start(out=wt[:],
                              in_=wslab[:].rearrange("(s p) d -> p s d", p=P))
            pool_t = pp.tile([P, 3], f16, name="pool_t", tag="pool_t")
            nc.sync.dma_start(out=pool_t[:], in_=pool_in[:])
            bias_t = {}
            for k, nz in (("bias_p1", bias_nz["c1w_bl"]),
                          ("bias_a1", bias_nz["c1b_bl"]),
                          ("bias_p2", bias_nz["skipP_b"]),
                          ("bias_a2", bias_nz["skipA_b"])):
                if nz:
                    t = pp.tile([P, H], f32, name=k + "_t")
                    nc.sync.dma_start(out=t[:], in_=bias_in[k][:])
                    bias_t[k] = t

            xaT = pp.tile([P, A_PAD], f16, name="xaT", tag="xaT")
            nc.sync.dma_start_transpose(out=xaT[:], in_=xa_chunk[:])
            xpT = pp.tile([P, P_PAD], f16, name="xpT", tag="xpT")
            nc.sync.dma_start_transpose(out=xpT[:], in_=xp_chunk[:])

            # h1 tables: fp8 local shard -> Shared-output AllGather table
            # (fp16 local shard feeds the DMA transposes for L2 root terms)
            h1a_sh = dp.tile([NA_AG, H], f8, name="h1a_sh", tag="h1a_sh",
                             addr_space="Shared")
            h1p_sh = dp.tile([NP_AG, H], f8, name="h1p_sh", tag="h1p_sh",
                             addr_space="Shared")
            h1a_l8 = dp.tile([A_PAD, H], f8, name="h1a_l8", tag="h1a_l8")
            h1p_l8 = dp.tile([P_PAD, H], f8, name="h1p_l8", tag="h1p_l8")
            h1a_loc = dp.tile([A_PAD, H], f16, name="h1a_loc", tag="h1a_loc")
            h1p_loc = dp.tile([P_PAD, H], f16, name="h1p_loc", tag="h1p_loc")

            def conv(nm, table, elem, Wl, Wr, rootT, skipW, skipT, bias,
                     h_l8, h_loc, pool_ps, pool_last_col):
                rl = rels[nm]
                nslice = elem // P
                it = idx_t[nm]
                mdt = f16 if nslice == 1 else f8
                gathers = []
                for w in range(rl.n_win):
                    wc = int(rl.wcols[w])
                    cb = int(rl.col_base[w])
                    aggT = []
                    if wc:
                        msgs = mp.tile([P, MAXW, elem], mdt,
                                       tag=f"msgs{nslice}")
                        for (b, ioff, nidx, lcb) in rl.ops[w]:
                            b0 = b * rl.bank_rows
                            b1 = min(b0 + rl.bank_rows, rl.table_rows)
                            gathers.append(nc.gpsimd.dma_gather(
                                msgs[:, lcb:lcb + nidx // P, :elem],
                                table[b0:b1, :],
                                it[:, ioff:ioff + nidx // 16],
                                nidx, nidx, elem, single_packet=False))
                        mask_t = mk.tile([P, MAXW * WD], f8, tag="mask")
                        nc.scalar.dma_start(
                            out=mask_t[:, :wc * WD],
                            in_=mask_h[nm][:, cb * WD:(cb + wc) * WD])
                        aggs = []
                        for s in range(nslice):
                            aggs.append(psA.tile([P, WD], f32, tag="agg",
                                                 name="agg", space="PSUM"))
                        for i in range(wc):
                            for s in range(nslice):
                                nc.tensor.matmul(
                                    out=aggs[s][:],
                                    lhsT=msgs[:, i:i + 1, s * P:(s + 1) * P],
                                    rhs=mask_t[:, i * WD:(i + 1) * WD],
                                    start=(i == 0), stop=(i == wc - 1))
                        for s in range(nslice):
                            a = wk.tile([P, WD], f16, tag="aggT")
                            nc.scalar.copy(out=a[:], in_=aggs[s][:])
                            aggT.append(a)
                    for tl in range(min(WIN, rl.n_tiles - w * WIN)):
                        t = w * WIN + tl
                        lin = psL.tile([P, H], f32, tag="lin", space="PSUM")
                        first = True
                        if wc:
                            for s in range(nslice):
                                nc.tensor.matmul(
                                    out=lin[:],
                                    lhsT=aggT[s][:, tl * P:(tl + 1) * P],
                                    rhs=wt[:, Wl[s]:Wl[s] + 1, :],
                                    start=first, stop=False)
                                first = False
                        for s in range(nslice):
                            nc.tensor.matmul(
                                out=lin[:],
                                lhsT=rootT[s][:, t * P:(t + 1) * P],
                                rhs=wt[:, Wr[s]:Wr[s] + 1, :],
                                start=first,
                                stop=(skipW is None and s == nslice - 1))
                            first = False
                        if skipW is not None:
                            nc.tensor.matmul(
                                out=lin[:], lhsT=skipT[:, t * P:(t + 1) * P],
                                rhs=wt[:, skipW:skipW + 1, :],
                                start=False, stop=True)
                        h16 = wk.tile([P, H], f16, tag="h16")
                        if bias is None:
                            src = lin
                        else:
                            tmp = wk.tile([P, H], f32, tag="btmp")
                            nc.vector.tensor_add(out=tmp[:], in0=lin[:],
                                                 in1=bias[:])
                            src = tmp
                        nc.scalar.activation(out=h16[:], in_=src[:],
                                             func=relu_f)
                        if h_l8 is not None:
                            h8 = wk.tile([P, H], f8, tag="h8")
                            nc.scalar.activation(out=h8[:], in_=src[:],
                                                 func=relu_f)
                            nc.scalar.dma_start(
                                out=h_l8[t * P:(t + 1) * P, :], in_=h8[:])
                            nc.scalar.dma_start(
                                out=h_loc[t * P:(t + 1) * P, :], in_=h16[:])
                        if pool_ps is not None:
                            oc = pool_last_col if t == rl.n_tiles - 1 else 0
                            nc.tensor.matmul(
                                out=pool_ps[:], lhsT=pool_t[:, oc:oc + 1],
                                rhs=h16[:], start=(t == 0),
                                stop=(t == rl.n_tiles - 1),
                                skip_group_check=True)
                return gathers

            # -------- layer 1: authors (wb: src papers -> dst authors)
            conv("B1", xp_cmp, IN, [W["c1b_Wl"]], [W["c1b_Wr"]], [xaT],
                 None, None, bias_t.get("bias_a1"), h1a_l8, h1a_loc, None, 0)
            h1aT = []
            for s in range(2):
                t = pp.tile([P, A_PAD], f16, name=f"h1aT{s}", tag=f"h1aT{s}")
                nc.sync.dma_start_transpose(
                    out=t[:], in_=h1a_loc[:, s * P:(s + 1) * P])
                h1aT.append(t)
            nc.gpsimd.collective_compute(
                "AllGather", mybir.AluOpType.bypass, replica_groups=rg,
                ins=[h1a_l8.opt()], outs=[h1a_sh.opt()])

            # -------- layer 1: papers (writes: src authors -> dst papers)
            conv("W1", xa_cmp, IN, [W["c1w_Wl"]], [W["c1w_Wr"]], [xpT],
                 None, None, bias_t.get("bias_p1"), h1p_l8, h1p_loc, None, 0)
            h1pT = []
            for s in range(2):
                t = pp.tile([P, P_PAD], f16, name=f"h1pT{s}", tag=f"h1pT{s}")
                nc.sync.dma_start_transpose(
                    out=t[:], in_=h1p_loc[:, s * P:(s + 1) * P])
                h1pT.append(t)

            # -------- layer 2: papers (gathers h1a from shared table)
            pool_p = psP.tile([1, H], f32, name="pool_p", tag="pool_p",
                              space="PSUM")
            pool_a = psP.tile([1, H], f32, name="pool_a", tag="pool_a",
                              space="PSUM")
            gW2 = conv("W2", h1a_sh, H, [W["c2w_Wl0"], W["c2w_Wl1"]],
                       [W["c2w_Wr0"], W["c2w_Wr1"]], h1pT, W["skipP_W"], xpT,
                       bias_t.get("bias_p2"), None, None, pool_p, 1)

            # AG(h1p): L2-papers does not consume it; pin it behind the last
            # L2-papers gather so the scheduler cannot hoist its inline wait
            # into the middle of the gather stream
            ccP = nc.gpsimd.collective_compute(
                "AllGather", mybir.AluOpType.bypass, replica_groups=rg,
                ins=[h1p_l8.opt()], outs=[h1p_sh.opt()])
            _add_dep_helper(ccP.ins, gW2[-1].ins,
                            reason="keep AG(h1p) after L2-papers gathers")

            # -------- layer 2: authors
            conv("B2", h1p_sh, H, [W["c2b_Wl0"], W["c2b_Wl1"]],
                 [W["c2b_Wr0"], W["c2b_Wr1"]], h1aT, W["skipA_W"], xaT,
                 bias_t.get("bias_a2"), None, None, pool_a, 2)

            pool_sb = wk.tile([1, 2 * H], f32, tag="poolout")
            nc.vector.tensor_copy(out=pool_sb[:, 0:H], in_=pool_a[:])
            nc.vector.tensor_copy(out=pool_sb[:, H:2 * H], in_=pool_p[:])
            nc.sync.dma_start(out=out_pool[:], in_=pool_sb[:])

            if debug:
                nc.sync.dma_start(out=dbg_h1a[:], in_=h1a_loc[:])
                nc.sync.dma_start(out=dbg_h1p[:], in_=h1p_loc[:])

    nc.compile()
    return nc


def kernel(**inputs):
    debug = bool(int(os.environ.get("GNN_DEBUG", "0")))
    trace = bool(int(os.environ.get("GNN_TRACE", "0")))
    rels, in_maps, bias_nz = _prep(inputs)
    nc = _build(rels, bias_nz, debug=debug)
    res = bass_utils.run_bass_kernel_spmd(
        nc, in_maps, core_ids=list(range(C)), trace=trace)
    kernel.last_results = res

    pools = np.stack([res.results[c]["out_pool"] for c in range(C)])
    sum_a = pools[:, 0, :H].astype(np.float64).sum(axis=0)
    sum_p = pools[:, 0, H:].astype(np.float64).sum(axis=0)
    pooled = np.concatenate([sum_a / NA, sum_p / NP_])[None, :]
    W1 = np.asarray(inputs["cls_W1"], np.float64)
    b1 = np.asarray(inputs["cls_b1"], np.float64)
    W2 = np.asarray(inputs["cls_W2"], np.float64)
    b2 = np.asarray(inputs["cls_b2"], np.float64)
    h = np.maximum(pooled @ W1.T + b1, 0.0)
    out = h @ W2.T + b2
    return out.astype(np.float32)


# revision 39
# speedup vs baseline: 1.0375x; 1.0375x over previous
"""Trainium2 Bass kernel for the GSAT HeteroGNN problem (8 NeuronCores).

Self-contained: hardcodes shapes/sharding; only imports the concourse
toolchain.

Strategy (dst-node sharding, SPMD over 8 cores):
  - papers split into 8 canonical chunks of 12500 (padded 12544 = 98 tiles),
    authors 8 x 6250 (padded 6272 = 49 tiles).
  - edges live on their dst's owner core, laid out host-side into 128-slot
    columns per (4-tile window, src-bank); dma_gather (int16 idx) fetches
    fp8 source rows as [128, cols, feat].
  - segment-mean via host-precomputed fp8 masks streamed by DMA:
    mask[slot, dst_in_window] = 1/deg(dst); TensorE accumulates
    aggT[feat, 512] in PSUM per window (no on-device mask building).
  - L1 gathers read per-core COMPACT fp8 tables (only the <=32k rows this
    core references -> single bank, minimal column padding).
  - L1 outputs h1 are written twice: fp8 rows into a local chunk that a
    Shared-output AllGather assembles into a shared fp8 table (each rank
    contributes only its 1.6-3.2MB shard; the old Local-output AllGathers
    moved 77MB/core), and fp16 into a local chunk used for DMA transposes
    (L2 root terms).
  - L2 gathers read the shared fp8 h1 tables directly.
  - all DMA transposes are placed before any collective in program order
    (the scheduler serializes transposes with collectives).
  - global mean-pool via ones-column matmuls accumulating in PSUM; final
    2-layer MLP on host in fp64.
"""
import os
import sys

try:
    import concourse  # noqa: F401
except ImportError:  # toolchain location in the grading container
    sys.path.insert(0, "/opt/trn_rl_repo")

import numpy as np
import ml_dtypes
from concourse import bass, bacc, mybir, tile  # noqa: F401
from concourse import bass_utils
from concourse.bass import _add_dep_helper

dt = mybir.dt
F8 = ml_dtypes.float8_e4m3

# ---------------------------------------------------------------- constants
NA, NP_, E = 50000, 100000, 300000
IN, H, OUT = 128, 256, 16
C = 8                      # cores
P = 128                    # partitions
A_CAN, P_CAN = NA // C, NP_ // C              # 6250 / 12500
A_PAD = ((A_CAN + P - 1) // P) * P            # 6272
P_PAD = ((P_CAN + P - 1) // P) * P            # 12544
NA_AG, NP_AG = C * A_PAD, C * P_PAD           # 50176 / 100352
WIN = 4                    # dst tiles per PSUM window (512 dsts)
WD = WIN * P               # window width in dsts


class RelLayer:
    """Host-side layout for one (relation, layer): slot columns per
    (window, bank), uniform across cores (max-over-cores column counts),
    int16 gather indices and fp8 recip masks."""

    def __init__(self, row_of, dst_owner, dstl, n_dst_can, n_dst_pad,
                 recip_dst_local, table_rows, parity=None):
        # parity: per-core per-edge 0/1 within its pair-row (paired mode);
        # row_of then holds PAIR ids and slots dedupe by (window, pair)
        # row_of: [C] list of per-edge row ids (into this layer's table)
        # dst_owner/dstl: per-edge owner core and local dst id (global arrays
        # already split: row_of[c] aligned with dstl[c])
        self.n_tiles = n_dst_pad // P
        self.n_win = (self.n_tiles + WIN - 1) // WIN
        nb = (table_rows + 32767) // 32768
        self.n_banks = nb
        self.bank_rows = (table_rows + nb - 1) // nb
        self.table_rows = table_rows

        self.paired = parity is not None
        # per-core per-cell counts -> uniform ncols
        ncols = np.zeros((self.n_win, nb), np.int64)
        per_core = []
        for c in range(C):
            rows, dl = row_of[c], dstl[c]
            par = parity[c] if self.paired else np.zeros(len(rows), np.int64)
            w = dl // WD
            b = rows // self.bank_rows
            if self.paired:
                # one slot per distinct (window, pair-row)
                key = w * self.table_rows + rows
                uk, inv = np.unique(key, return_inverse=True)
                cnt = np.zeros((self.n_win, nb), np.int64)
                np.add.at(cnt, ((uk // self.table_rows),
                                (uk % self.table_rows) // self.bank_rows), 1)
            else:
                inv = None
                cnt = np.zeros((self.n_win, nb), np.int64)
                np.add.at(cnt, (w, b), 1)
            ncols = np.maximum(ncols, (cnt + P - 1) // P)
            per_core.append((rows, dl, w, b, par, inv))
        self.ncols = ncols

        # global column layout: window-major, bank-minor
        self.col_base = np.zeros(self.n_win + 1, np.int64)
        self.ops = []              # per window: list of (bank, ioff, nidx, lcb)
        ioff = 0
        col = 0
        for w in range(self.n_win):
            self.col_base[w] = col
            wops = []
            lcb = 0
            for b in range(nb):
                nco = int(ncols[w, b])
                if nco:
                    wops.append((b, ioff, nco * P, lcb))
                    ioff += nco * P // 16
                    lcb += nco
                    col += nco
            self.ops.append(wops)
        self.col_base[self.n_win] = col
        self.total_cols = col
        self.idx_width = ioff
        self.wcols = np.diff(self.col_base).astype(np.int64)
        self.max_wcols = int(self.wcols.max()) if col else 0
        self.total_idx = col * P

        # per-core idx + masks
        self.idx16 = np.zeros((C, P, max(self.idx_width, 1)), np.int16)
        self.masks = np.zeros((C, P, max(col, 1), WD), F8)  # repl. if paired
        cell_base = {}
        lcb_map = {}
        for w in range(self.n_win):
            for (b, io, nidx, lcb) in self.ops[w]:
                cell_base[(w, b)] = io
                lcb_map[(w, b)] = self.col_base[w] + lcb
        mw = 2 if self.paired else 1
        self.masks = np.zeros((C, P, max(col, 1) * mw, WD), F8)
        for c in range(C):
            rows, dl, w_e, b_e, par_e, inv = per_core[c]
            if self.paired:
                # slot per distinct (w, pair): compute slot ranks per cell
                key_sl = w_e * self.table_rows + rows
                uk = np.unique(key_sl)
                sw = uk // self.table_rows
                srow = uk % self.table_rows
                sb = srow // self.bank_rows
                cellkey = sw * nb + sb
                cellcnt = np.bincount(cellkey, minlength=self.n_win * nb)
                starts = np.zeros(self.n_win * nb + 1, np.int64)
                np.cumsum(cellcnt, out=starts[1:])
                jslot = np.arange(len(uk)) - starts[cellkey]
                # idx slab: one idx per slot
                flat = np.zeros(max(self.idx_width, 1) * 16, np.int16)
                iobase = np.array([cell_base.get((w, b), -1) * 16
                                   for w in range(self.n_win)
                                   for b in range(nb)]).reshape(self.n_win, nb)
                flat[iobase[sw, sb] + jslot] = (srow % self.bank_rows
                                                ).astype(np.int16)
                w16 = flat.reshape(-1, 16).T
                self.idx16[c] = np.tile(w16, (8, 1))
                # masks: edges land at their slot, plane = parity
                gcol = np.array([lcb_map.get((w, b), 0)
                                 for w in range(self.n_win)
                                 for b in range(nb)]).reshape(self.n_win, nb)
                slot_of_edge = np.searchsorted(uk, key_sl)
                je = jslot[slot_of_edge]
                cc = gcol[sw[slot_of_edge], sb[slot_of_edge]] + je // P
                pp = je % P
                off = dl - w_e * WD
                rec = recip_dst_local[c][dl].astype(np.float32)
                mbuf = np.zeros((P, max(col, 1) * mw, WD), np.float32)
                np.add.at(mbuf, (pp, cc * 2 + par_e, off), rec)
                self.masks[c] = mbuf.astype(F8)
                continue
            order = np.argsort(w_e * nb + b_e, kind="stable")
            rows_s, dl_s, w_s, b_s = rows[order], dl[order], w_e[order], b_e[order]
            rec_s = recip_dst_local[c][dl_s].astype(np.float32)
            rib_s = (rows_s % self.bank_rows).astype(np.int64)
            # rank within each (w, b) run
            key = w_s * nb + b_s
            # j = index within cell
            cellcnt = np.bincount(key, minlength=self.n_win * nb)
            starts = np.zeros(self.n_win * nb + 1, np.int64)
            np.cumsum(cellcnt, out=starts[1:])
            j = np.arange(len(key)) - starts[key]
            # idx slab (flat over ops)
            flat = np.zeros(max(self.idx_width, 1) * 16, np.int16)
            iobase = np.array([cell_base.get((w, b), -1) * 16
                               for w in range(self.n_win) for b in range(nb)]
                              ).reshape(self.n_win, nb)
            pos = iobase[w_s, b_s] + j
            flat[pos] = rib_s.astype(np.int16)
            w16 = flat.reshape(-1, 16).T       # [16, width]
            self.idx16[c] = np.tile(w16, (8, 1))
            # masks
            gcol = np.array([lcb_map.get((w, b), 0)
                             for w in range(self.n_win) for b in range(nb)]
                            ).reshape(self.n_win, nb)
            cc = gcol[w_s, b_s] + j // P
            pp = j % P
            off = dl_s - w_s * WD
            self.masks[c][pp, cc, off] = rec_s.astype(F8)


def _pair_srcs(srcs, dstls):
    """Per-core greedy pairing of gather sources by co-window occurrence.
    Returns per-core (pair_id per edge, parity per edge, pair row list)."""
    out = []
    for c in range(C):
        s, dl = srcs[c], dstls[c]
        w = dl // WD
        partner = {}
        for wi in range(int(w.max()) + 1):
            ss = np.unique(s[w == wi])
            free = [int(x) for x in ss if int(x) not in partner]
            for a, b in zip(free[0::2], free[1::2]):
                partner[a] = b
                partner[b] = a
        uniq = np.unique(s)
        left = [int(x) for x in uniq if int(x) not in partner]
        for a, b in zip(left[0::2], left[1::2]):
            partner[a] = b
            partner[b] = a
        if len(left) % 2:
            partner[left[-1]] = -1
        pair_rows = []
        pair_of, parity_of = {}, {}
        for x in uniq:
            x = int(x)
            if x in pair_of:
                continue
            p = partner[x]
            k = len(pair_rows)
            if p == -1:
                pair_rows.append((x, x))
                pair_of[x] = k
                parity_of[x] = 0
            else:
                pair_rows.append((x, p))
                pair_of[x] = k
                parity_of[x] = 0
                pair_of[p] = k
                parity_of[p] = 1
        ids = np.array([pair_of[int(x)] for x in s], np.int64)
        par = np.array([parity_of[int(x)] for x in s], np.int64)
        out.append((ids, par, pair_rows))
    return out


def _balance_perm(deg, n_nodes, can):
    """Permutation node -> new global id, dealing nodes into (core, window)
    cells so per-cell degree sums are balanced (pool is perm-invariant)."""
    import heapq
    n_win = ((can + P - 1) // P + WIN - 1) // WIN
    caps, base = [], []
    for c in range(C):
        for w in range(n_win):
            cap = min(WD, can - w * WD)
            caps.append(cap)
            base.append(c * can + w * WD)
    order = np.argsort(-deg, kind="stable")
    heap = [(0.0, i) for i in range(len(caps))]
    heapq.heapify(heap)
    fill = np.zeros(len(caps), np.int64)
    perm = np.empty(n_nodes, np.int64)
    for nd in order:
        while True:
            s, i = heapq.heappop(heap)
            if fill[i] < caps[i]:
                break
        perm[nd] = base[i] + fill[i]
        fill[i] += 1
        if fill[i] < caps[i]:
            heapq.heappush(heap, (s + deg[nd], i))
    return perm


def _prep(inputs):
    f = lambda k: np.asarray(inputs[k], np.float32)
    x_author, x_paper = f("x_author"), f("x_paper")
    ws, wd = (np.asarray(inputs["ei_writes_src"], np.int64),
              np.asarray(inputs["ei_writes_dst"], np.int64))
    bs, bd = (np.asarray(inputs["ei_wb_src"], np.int64),
              np.asarray(inputs["ei_wb_dst"], np.int64))

    # relabel nodes so per-(core, window) dst-degree sums are balanced
    pa_perm = _balance_perm(np.bincount(wd, minlength=NP_), NP_, P_CAN)
    au_perm = _balance_perm(np.bincount(bd, minlength=NA), NA, A_CAN)
    inv_pa = np.empty(NP_, np.int64)
    inv_pa[pa_perm] = np.arange(NP_)
    inv_au = np.empty(NA, np.int64)
    inv_au[au_perm] = np.arange(NA)
    x_paper = x_paper[inv_pa]
    x_author = x_author[inv_au]
    wd, bs = pa_perm[wd], pa_perm[bs]
    ws, bd = au_perm[ws], au_perm[bd]

    cnt_p = np.bincount(wd, minlength=NP_).astype(np.float32)
    cnt_a = np.bincount(bd, minlength=NA).astype(np.float32)
    recip_p = 1.0 / np.maximum(cnt_p, 1.0)
    recip_a = 1.0 / np.maximum(cnt_a, 1.0)

    # split edges by dst owner
    def split(src, dst, dst_can):
        srcs, dstls = [], []
        for c in range(C):
            m = (dst // dst_can) == c
            srcs.append(src[m])
            dstls.append((dst[m] % dst_can).astype(np.int64))
        return srcs, dstls

    w_src, w_dstl = split(ws, wd, P_CAN)     # writes: dst papers
    b_src, b_dstl = split(bs, bd, A_CAN)     # wb: dst authors

    # L1 compact PAIR tables: two co-window srcs per 256B fp8 row, so one
    # gather descriptor serves up to two edges
    pairsW = _pair_srcs(w_src, w_dstl)       # authors referenced per core
    pairsB = _pair_srcs(b_src, b_dstl)       # papers referenced per core
    rowsW = ((max(len(p[2]) for p in pairsW) + P - 1) // P) * P
    rowsB = ((max(len(p[2]) for p in pairsB) + P - 1) // P) * P
    assert rowsW <= 32768 and rowsB <= 32768
    xa_cmp = np.zeros((C, rowsW, 2 * IN), F8)
    xp_cmp = np.zeros((C, rowsB, 2 * IN), F8)
    for c in range(C):
        pr = np.array(pairsW[c][2], np.int64)
        xa_cmp[c, :len(pr), :IN] = x_author[pr[:, 0]].astype(F8)
        xa_cmp[c, :len(pr), IN:] = x_author[pr[:, 1]].astype(F8)
        pr = np.array(pairsB[c][2], np.int64)
        xp_cmp[c, :len(pr), :IN] = x_paper[pr[:, 0]].astype(F8)
        xp_cmp[c, :len(pr), IN:] = x_paper[pr[:, 1]].astype(F8)

    recip_p_loc = [recip_p[c * P_CAN:(c + 1) * P_CAN] for c in range(C)]
    recip_a_loc = [recip_a[c * A_CAN:(c + 1) * A_CAN] for c in range(C)]

    # AG row mapping for L2 tables
    agW = [(s // A_CAN) * A_PAD + (s % A_CAN) for s in w_src]
    agB = [(s // P_CAN) * P_PAD + (s % P_CAN) for s in b_src]
    rels = dict(
        W1=RelLayer([p[0] for p in pairsW], None, w_dstl, P_CAN, P_PAD,
                    recip_p_loc, rowsW, parity=[p[1] for p in pairsW]),
        B1=RelLayer([p[0] for p in pairsB], None, b_dstl, A_CAN, A_PAD,
                    recip_a_loc, rowsB, parity=[p[1] for p in pairsB]),
        W2=RelLayer(agW, None, w_dstl, P_CAN, P_PAD, recip_p_loc, NA_AG),
        B2=RelLayer(agB, None, b_dstl, A_CAN, A_PAD, recip_a_loc, NP_AG),
    )

    # fp16 local chunks (root/skip transposes)
    xa_chunk = np.zeros((C, A_PAD, IN), np.float16)
    xp_chunk = np.zeros((C, P_PAD, IN), np.float16)
    for c in range(C):
        xa_chunk[c, :A_CAN] = x_author[c * A_CAN:(c + 1) * A_CAN]
        xp_chunk[c, :P_CAN] = x_paper[c * P_CAN:(c + 1) * P_CAN]

    # weight slab: 14 x [128, 256] fp16 (transposed: [in, out])
    wT = lambda k: f(k).T.astype(np.float16)
    slabs = [wT("c1w_Wl"), wT("c1w_Wr"), wT("c1b_Wl"), wT("c1b_Wr")]
    for k in ("c2w_Wl", "c2w_Wr", "c2b_Wl", "c2b_Wr"):
        w2 = wT(k)
        slabs += [w2[:128], w2[128:]]
    slabs += [wT("skipA_W"), wT("skipP_W")]
    wslab = np.concatenate(slabs, axis=0)          # [14*128, 256]

    pool_ones = np.zeros((P, 3), np.float16)
    pool_ones[:, 0] = 1.0
    pool_ones[:P_CAN - (P_PAD // P - 1) * P, 1] = 1.0   # last paper tile mask
    pool_ones[:A_CAN - (A_PAD // P - 1) * P, 2] = 1.0   # last author tile mask

    bias_nz = {k: bool(np.any(f(k))) for k in
               ("c1w_bl", "c1b_bl", "skipA_b", "skipP_b")}
    bias_arr = {k: np.broadcast_to(f(k2), (P, H)).astype(np.float32).copy()
                for k, k2 in (("bias_p1", "c1w_bl"), ("bias_a1", "c1b_bl"),
                              ("bias_p2", "skipP_b"), ("bias_a2", "skipA_b"))}

    in_maps = []
    for c in range(C):
        m = dict(
            xa_cmp=xa_cmp[c], xp_cmp=xp_cmp[c],
            xa_chunk=xa_chunk[c], xp_chunk=xp_chunk[c],
            wslab=wslab, pool_ones=pool_ones,
        )
        for nm, rl in rels.items():
            m["idx_" + nm] = rl.idx16[c]
            m["mask_" + nm] = rl.masks[c].reshape(P, -1)
        for k, arr in bias_arr.items():
            m[k] = arr
        in_maps.append(m)
    return rels, in_maps, bias_nz


def _build(rels, bias_nz, debug=False):
    nc = bacc.Bacc("TRN2", target_bir_lowering=False, debug=False,
                   num_devices=C)
    f16, f32, i16, f8 = dt.float16, dt.float32, dt.int16, dt.float8e4
    ein = lambda n, s, d: nc.dram_tensor(n, s, d, kind="ExternalInput")

    xa_cmp = ein("xa_cmp", [rels["W1"].table_rows, 2 * IN], f8)
    xp_cmp = ein("xp_cmp", [rels["B1"].table_rows, 2 * IN], f8)
    xa_chunk = ein("xa_chunk", [A_PAD, IN], f16)
    xp_chunk = ein("xp_chunk", [P_PAD, IN], f16)
    wslab = ein("wslab", [14 * P, H], f16)
    pool_in = ein("pool_ones", [P, 3], f16)
    idx_h, mask_h = {}, {}
    for nm, rl in rels.items():
        mw = 2 if rl.paired else 1
        idx_h[nm] = ein("idx_" + nm, [P, max(rl.idx_width, 1)], i16)
        mask_h[nm] = ein("mask_" + nm, [P, max(rl.total_cols, 1) * mw * WD], f8)
    bias_in = {k: ein(k, [P, H], f32)
               for k in ("bias_p1", "bias_a1", "bias_p2", "bias_a2")}

    out_pool = nc.dram_tensor("out_pool", [1, 2 * H], f32,
                              kind="ExternalOutput")
    if debug:
        dbg_h1a = nc.dram_tensor("dbg_h1a", [A_PAD, H], f16,
                                 kind="ExternalOutput")
        dbg_h1p = nc.dram_tensor("dbg_h1p", [P_PAD, H], f16,
                                 kind="ExternalOutput")

    W = {k: i for i, k in enumerate(
        ["c1w_Wl", "c1w_Wr", "c1b_Wl", "c1b_Wr",
         "c2w_Wl0", "c2w_Wl1", "c2w_Wr0", "c2w_Wr1",
         "c2b_Wl0", "c2b_Wl1", "c2b_Wr0", "c2b_Wr1",
         "skipA_W", "skipP_W"])}
    relu_f = mybir.ActivationFunctionType.Relu
    rg = [list(range(C))]
    MAXW = max(rl.max_wcols for rl in rels.values())
    MAXM = max(rl.max_wcols * (2 if rl.paired else 1) for rl in rels.values())

    with tile.TileContext(nc) as tc:
        with tc.tile_pool(name="persist", bufs=1) as pp, \
             tc.tile_pool(name="dram", bufs=1, space="DRAM") as dp, \
             tc.tile_pool(name="work", bufs=3) as wk, \
             tc.tile_pool(name="msgs", bufs=2) as mp, \
             tc.tile_pool(name="maskp", bufs=2) as mk, \
             tc.tile_pool(name="psA", bufs=4, space="PSUM") as psA, \
             tc.tile_pool(name="psL", bufs=2, space="PSUM") as psL, \
             tc.tile_pool(name="psP", bufs=1, space="PSUM") as psP:

            # ---------------- persistent loads (idx first: gathers need it)
            idx_t = {}
            for nm in ("B1", "W1", "W2", "B2"):
                rl = rels[nm]
                t = pp.tile([P, max(rl.idx_width, 1)], i16, name="idx" + nm)
                nc.sync.dma_start(out=t[:], in_=idx_h[nm][:])
                idx_t[nm] = t
            wt = pp.tile([P, 14, H], f16, name="wt", tag="wt")
            nc.sync.dma_start(out=wt[:],
                              in_=wslab[:].rearrange("(s p) d -> p s d", p=P))
            pool_t = pp.tile([P, 3], f16, name="pool_t", tag="pool_t")
            nc.sync.dma_start(out=pool_t[:], in_=pool_in[:])
            bias_t = {}
            for k, nz in (("bias_p1", bias_nz["c1w_bl"]),
                          ("bias_a1", bias_nz["c1b_bl"]),
                          ("bias_p2", bias_nz["skipP_b"]),
                          ("bias_a2", bias_nz["skipA_b"])):
                if nz:
                    t = pp.tile([P, H], f32, name=k + "_t")
                    nc.sync.dma_start(out=t[:], in_=bias_in[k][:])
                    bias_t[k] = t

            xaT = pp.tile([P, A_PAD], f16, name="xaT", tag="xaT")
            nc.sync.dma_start_transpose(out=xaT[:], in_=xa_chunk[:])
            xpT = pp.tile([P, P_PAD], f16, name="xpT", tag="xpT")
            nc.sync.dma_start_transpose(out=xpT[:], in_=xp_chunk[:])

            # h1 tables: fp8 local shard -> Shared-output AllGather table
            # (fp16 local shard feeds the DMA transposes for L2 root terms)
            h1a_sh = dp.tile([NA_AG, H], f8, name="h1a_sh", tag="h1a_sh",
                             addr_space="Shared")
            h1p_sh = dp.tile([NP_AG, H], f8, name="h1p_sh", tag="h1p_sh",
                             addr_space="Shared")
            h1a_l8 = dp.tile([A_PAD, H], f8, name="h1a_l8", tag="h1a_l8")
            h1p_l8 = dp.tile([P_PAD, H], f8, name="h1p_l8", tag="h1p_l8")
            h1a_loc = dp.tile([A_PAD, H], f16, name="h1a_loc", tag="h1a_loc")
            h1p_loc = dp.tile([P_PAD, H], f16, name="h1p_loc", tag="h1p_loc")

            def conv(nm, table, elem, Wl, Wr, rootT, skipW, skipT, bias,
                     h_l8, h_loc, pool_ps, pool_last_col):
                rl = rels[nm]
                nslice = elem // P
                it = idx_t[nm]
                fetch = 2 * elem if rl.paired else elem
                mw = 2 if rl.paired else 1
                gathers = []
                for w in range(rl.n_win):
                    wc = int(rl.wcols[w])
                    cb = int(rl.col_base[w])
                    aggT = []
                    if wc:
                        msgs = mp.tile([P, MAXW, 256], f8, tag="msgs")
                        for (b, ioff, nidx, lcb) in rl.ops[w]:
                            b0 = b * rl.bank_rows
                            b1 = min(b0 + rl.bank_rows, rl.table_rows)
                            gathers.append(nc.gpsimd.dma_gather(
                                msgs[:, lcb:lcb + nidx // P, :fetch],
                                table[b0:b1, :],
                                it[:, ioff:ioff + nidx // 16],
                                nidx, nidx, fetch, single_packet=False))
                        mask_t = mk.tile([P, MAXM * WD], f8, tag="mask")
                        nc.scalar.dma_start(
                            out=mask_t[:, :wc * mw * WD],
                            in_=mask_h[nm][:, cb * mw * WD:
                                           (cb + wc) * mw * WD])
                        aggs = []
                        for s in range(nslice):
                            aggs.append(psA.tile([P, WD], f32, tag="agg",
                                                 name="agg", space="PSUM"))
                        for i in range(wc):
                            for h in range(mw):
                                for s in range(nslice):
                                    nc.tensor.matmul(
                                        out=aggs[s][:],
                                        lhsT=msgs[:, i:i + 1,
                                                  (h * nslice + s) * P:
                                                  (h * nslice + s + 1) * P],
                                        rhs=mask_t[:, (mw * i + h) * WD:
                                                   (mw * i + h + 1) * WD],
                                        start=(i == 0 and h == 0),
                                        stop=(i == wc - 1 and h == mw - 1))
                        for s in range(nslice):
                            a = wk.tile([P, WD], f16, tag="aggT")
                            nc.scalar.copy(out=a[:], in_=aggs[s][:])
                            aggT.append(a)
                    for tl in range(min(WIN, rl.n_tiles - w * WIN)):
                        t = w * WIN + tl
                        lin = psL.tile([P, H], f32, tag="lin", space="PSUM")
                        first = True
                        if wc:
                            for s in range(nslice):
                                nc.tensor.matmul(
                                    out=lin[:],
                                    lhsT=aggT[s][:, tl * P:(tl + 1) * P],
                                    rhs=wt[:, Wl[s]:Wl[s] + 1, :],
                                    start=first, stop=False)
                                first = False
                        for s in range(nslice):
                            nc.tensor.matmul(
                                out=lin[:],
                                lhsT=rootT[s][:, t * P:(t + 1) * P],
                                rhs=wt[:, Wr[s]:Wr[s] + 1, :],
                                start=first,
                                stop=(skipW is None and s == nslice - 1))
                            first = False
                        if skipW is not None:
                            nc.tensor.matmul(
                                out=lin[:], lhsT=skipT[:, t * P:(t + 1) * P],
                                rhs=wt[:, skipW:skipW + 1, :],
                                start=False, stop=True)
                        h16 = wk.tile([P, H], f16, tag="h16")
                        if bias is None:
                            src = lin
                        else:
                            tmp = wk.tile([P, H], f32, tag="btmp")
                            nc.vector.tensor_add(out=tmp[:], in0=lin[:],
                                                 in1=bias[:])
                            src = tmp
                        nc.scalar.activation(out=h16[:], in_=src[:],
                                             func=relu_f)
                        if h_l8 is not None:
                            h8 = wk.tile([P, H], f8, tag="h8")
                            nc.scalar.activation(out=h8[:], in_=src[:],
                                                 func=relu_f)
                            nc.scalar.dma_start(
                                out=h_l8[t * P:(t + 1) * P, :], in_=h8[:])
                            nc.scalar.dma_start(
                                out=h_loc[t * P:(t + 1) * P, :], in_=h16[:])
                        if pool_ps is not None:
                            oc = pool_last_col if t == rl.n_tiles - 1 else 0
                            nc.tensor.matmul(
                                out=pool_ps[:], lhsT=pool_t[:, oc:oc + 1],
                                rhs=h16[:], start=(t == 0),
                                stop=(t == rl.n_tiles - 1),
                                skip_group_check=True)
                return gathers

            # -------- layer 1: authors (wb: src papers -> dst authors)
            conv("B1", xp_cmp, IN, [W["c1b_Wl"]], [W["c1b_Wr"]], [xaT],
                 None, None, bias_t.get("bias_a1"), h1a_l8, h1a_loc, None, 0)
            h1aT = []
            for s in range(2):
                t = pp.tile([P, A_PAD], f16, name=f"h1aT{s}", tag=f"h1aT{s}")
                nc.sync.dma_start_transpose(
                    out=t[:], in_=h1a_loc[:, s * P:(s + 1) * P])
                h1aT.append(t)
            nc.gpsimd.collective_compute(
                "AllGather", mybir.AluOpType.bypass, replica_groups=rg,
                ins=[h1a_l8.opt()], outs=[h1a_sh.opt()])

            # -------- layer 1: papers (writes: src authors -> dst papers)
            conv("W1", xa_cmp, IN, [W["c1w_Wl"]], [W["c1w_Wr"]], [xpT],
                 None, None, bias_t.get("bias_p1"), h1p_l8, h1p_loc, None, 0)
            h1pT = []
            for s in range(2):
                t = pp.tile([P, P_PAD], f16, name=f"h1pT{s}", tag=f"h1pT{s}")
                nc.sync.dma_start_transpose(
                    out=t[:], in_=h1p_loc[:, s * P:(s + 1) * P])
                h1pT.append(t)

            # -------- layer 2: papers (gathers h1a from shared table)
            pool_p = psP.tile([1, H], f32, name="pool_p", tag="pool_p",
                              space="PSUM")
            pool_a = psP.tile([1, H], f32, name="pool_a", tag="pool_a",
                              space="PSUM")
            gW2 = conv("W2", h1a_sh, H, [W["c2w_Wl0"], W["c2w_Wl1"]],
                       [W["c2w_Wr0"], W["c2w_Wr1"]], h1pT, W["skipP_W"], xpT,
                       bias_t.get("bias_p2"), None, None, pool_p, 1)

            # AG(h1p): L2-papers does not consume it; pin it behind the last
            # L2-papers gather so the scheduler cannot hoist its inline wait
            # into the middle of the gather stream
            ccP = nc.gpsimd.collective_compute(
                "AllGather", mybir.AluOpType.bypass, replica_groups=rg,
                ins=[h1p_l8.opt()], outs=[h1p_sh.opt()])
            _add_dep_helper(ccP.ins, gW2[-1].ins,
                            reason="keep AG(h1p) after L2-papers gathers")

            # -------- layer 2: authors
            conv("B2", h1p_sh, H, [W["c2b_Wl0"], W["c2b_Wl1"]],
                 [W["c2b_Wr0"], W["c2b_Wr1"]], h1aT, W["skipA_W"], xaT,
                 bias_t.get("bias_a2"), None, None, pool_a, 2)

            pool_sb = wk.tile([1, 2 * H], f32, tag="poolout")
            nc.vector.tensor_copy(out=pool_sb[:, 0:H], in_=pool_a[:])
            nc.vector.tensor_copy(out=pool_sb[:, H:2 * H], in_=pool_p[:])
            nc.sync.dma_start(out=out_pool[:], in_=pool_sb[:])

            if debug:
                nc.sync.dma_start(out=dbg_h1a[:], in_=h1a_loc[:])
                nc.sync.dma_start(out=dbg_h1p[:], in_=h1p_loc[:])

    nc.compile()
    return nc


def kernel(**inputs):
    debug = bool(int(os.environ.get("GNN_DEBUG", "0")))
    trace = bool(int(os.environ.get("GNN_TRACE", "0")))
    rels, in_maps, bias_nz = _prep(inputs)
    nc = _build(rels, bias_nz, debug=debug)
    res = bass_utils.run_bass_kernel_spmd(
        nc, in_maps, core_ids=list(range(C)), trace=trace)
    kernel.last_results = res

    pools = np.stack([res.results[c]["out_pool"] for c in range(C)])
    sum_a = pools[:, 0, :H].astype(np.float64).sum(axis=0)
    sum_p = pools[:, 0, H:].astype(np.float64).sum(axis=0)
    pooled = np.concatenate([sum_a / NA, sum_p / NP_])[None, :]
    W1 = np.asarray(inputs["cls_W1"], np.float64)
    b1 = np.asarray(inputs["cls_b1"], np.float64)
    W2 = np.asarray(inputs["cls_W2"], np.float64)
    b2 = np.asarray(inputs["cls_b2"], np.float64)
    h = np.maximum(pooled @ W1.T + b1, 0.0)
    out = h @ W2.T + b2
    return out.astype(np.float32)


# revision 40
# speedup vs baseline: 1.0680x; 1.0295x over previous
"""Trainium2 Bass kernel for the GSAT HeteroGNN problem (8 NeuronCores).

Self-contained: hardcodes shapes/sharding; only imports the concourse
toolchain.

Strategy (dst-node sharding, SPMD over 8 cores):
  - papers split into 8 canonical chunks of 12500 (padded 12544 = 98 tiles),
    authors 8 x 6250 (padded 6272 = 49 tiles).
  - edges live on their dst's owner core, laid out host-side into 128-slot
    columns per (4-tile window, src-bank); dma_gather (int16 idx) fetches
    fp8 source rows as [128, cols, feat].
  - segment-mean via host-precomputed fp8 masks streamed by DMA:
    mask[slot, dst_in_window] = 1/deg(dst); TensorE accumulates
    aggT[feat, 512] in PSUM per window (no on-device mask building).
  - L1 gathers read per-core COMPACT fp8 tables (only the <=32k rows this
    core references -> single bank, minimal column padding).
  - L1 outputs h1 are written twice: fp8 rows into a local chunk that a
    Shared-output AllGather assembles into a shared fp8 table (each rank
    contributes only its 1.6-3.2MB shard; the old Local-output AllGathers
    moved 77MB/core), and fp16 into a local chunk used for DMA transposes
    (L2 root terms).
  - L2 gathers read the shared fp8 h1 tables directly.
  - all DMA transposes are placed before any collective in program order
    (the scheduler serializes transposes with collectives).
  - global mean-pool via ones-column matmuls accumulating in PSUM; final
    2-layer MLP on host in fp64.
"""
import os
import sys

try:
    import concourse  # noqa: F401
except ImportError:  # toolchain location in the grading container
    sys.path.insert(0, "/opt/trn_rl_repo")

import numpy as np
import ml_dtypes
from concourse import bass, bacc, mybir, tile  # noqa: F401
from concourse import bass_utils
from concourse.bass import _add_dep_helper

dt = mybir.dt
F8 = ml_dtypes.float8_e4m3

# ---------------------------------------------------------------- constants
NA, NP_, E = 50000, 100000, 300000
IN, H, OUT = 128, 256, 16
C = 8                      # cores
P = 128                    # partitions
A_CAN, P_CAN = NA // C, NP_ // C              # 6250 / 12500
A_PAD = ((A_CAN + P - 1) // P) * P            # 6272
P_PAD = ((P_CAN + P - 1) // P) * P            # 12544
NA_AG, NP_AG = C * A_PAD, C * P_PAD           # 50176 / 100352
WIN = 4                    # dst tiles per PSUM window (512 dsts)
WD = WIN * P               # window width in dsts


class RelLayer:
    """Host-side layout for one (relation, layer): slot columns per
    (window, bank), uniform across cores (max-over-cores column counts),
    int16 gather indices and fp8 recip masks."""

    def __init__(self, row_of, dst_owner, dstl, n_dst_can, n_dst_pad,
                 recip_dst_local, table_rows, parity=None):
        # parity: per-core per-edge 0/1 within its pair-row (paired mode);
        # row_of then holds PAIR ids and slots dedupe by (window, pair)
        # row_of: [C] list of per-edge row ids (into this layer's table)
        # dst_owner/dstl: per-edge owner core and local dst id (global arrays
        # already split: row_of[c] aligned with dstl[c])
        self.n_tiles = n_dst_pad // P
        self.n_win = (self.n_tiles + WIN - 1) // WIN
        nb = (table_rows + 32767) // 32768
        self.n_banks = nb
        self.bank_rows = (table_rows + nb - 1) // nb
        self.table_rows = table_rows

        self.paired = parity is not None
        # per-core per-cell counts -> uniform ncols
        ncols = np.zeros((self.n_win, nb), np.int64)
        per_core = []
        for c in range(C):
            rows, dl = row_of[c], dstl[c]
            par = parity[c] if self.paired else np.zeros(len(rows), np.int64)
            w = dl // WD
            b = rows // self.bank_rows
            if self.paired:
                # one slot per distinct (window, pair-row)
                key = w * self.table_rows + rows
                uk, inv = np.unique(key, return_inverse=True)
                cnt = np.zeros((self.n_win, nb), np.int64)
                np.add.at(cnt, ((uk // self.table_rows),
                                (uk % self.table_rows) // self.bank_rows), 1)
            else:
                inv = None
                cnt = np.zeros((self.n_win, nb), np.int64)
                np.add.at(cnt, (w, b), 1)
            ncols = np.maximum(ncols, (cnt + P - 1) // P)
            per_core.append((rows, dl, w, b, par, inv))
        self.ncols = ncols

        # global column layout: window-major, bank-minor
        self.col_base = np.zeros(self.n_win + 1, np.int64)
        self.ops = []              # per window: list of (bank, ioff, nidx, lcb)
        ioff = 0
        col = 0
        for w in range(self.n_win):
            self.col_base[w] = col
            wops = []
            lcb = 0
            for b in range(nb):
                nco = int(ncols[w, b])
                if nco:
                    wops.append((b, ioff, nco * P, lcb))
                    ioff += nco * P // 16
                    lcb += nco
                    col += nco
            self.ops.append(wops)
        self.col_base[self.n_win] = col
        self.total_cols = col
        self.idx_width = ioff
        self.wcols = np.diff(self.col_base).astype(np.int64)
        self.max_wcols = int(self.wcols.max()) if col else 0
        self.total_idx = col * P

        # per-core idx + masks
        self.idx16 = np.zeros((C, P, max(self.idx_width, 1)), np.int16)
        self.masks = np.zeros((C, P, max(col, 1), WD), F8)  # repl. if paired
        cell_base = {}
        lcb_map = {}
        for w in range(self.n_win):
            for (b, io, nidx, lcb) in self.ops[w]:
                cell_base[(w, b)] = io
                lcb_map[(w, b)] = self.col_base[w] + lcb
        mw = 2 if self.paired else 1
        self.masks = np.zeros((C, P, max(col, 1) * mw, WD), F8)
        for c in range(C):
            rows, dl, w_e, b_e, par_e, inv = per_core[c]
            if self.paired:
                # slot per distinct (w, pair): compute slot ranks per cell
                key_sl = w_e * self.table_rows + rows
                uk = np.unique(key_sl)
                sw = uk // self.table_rows
                srow = uk % self.table_rows
                sb = srow // self.bank_rows
                cellkey = sw * nb + sb
                cellcnt = np.bincount(cellkey, minlength=self.n_win * nb)
                starts = np.zeros(self.n_win * nb + 1, np.int64)
                np.cumsum(cellcnt, out=starts[1:])
                jslot = np.arange(len(uk)) - starts[cellkey]
                # idx slab: one idx per slot
                flat = np.zeros(max(self.idx_width, 1) * 16, np.int16)
                iobase = np.array([cell_base.get((w, b), -1) * 16
                                   for w in range(self.n_win)
                                   for b in range(nb)]).reshape(self.n_win, nb)
                flat[iobase[sw, sb] + jslot] = (srow % self.bank_rows
                                                ).astype(np.int16)
                w16 = flat.reshape(-1, 16).T
                self.idx16[c] = np.tile(w16, (8, 1))
                # masks: edges land at their slot, plane = parity
                gcol = np.array([lcb_map.get((w, b), 0)
                                 for w in range(self.n_win)
                                 for b in range(nb)]).reshape(self.n_win, nb)
                slot_of_edge = np.searchsorted(uk, key_sl)
                je = jslot[slot_of_edge]
                cc = gcol[sw[slot_of_edge], sb[slot_of_edge]] + je // P
                pp = je % P
                off = dl - w_e * WD
                rec = recip_dst_local[c][dl].astype(np.float32)
                mbuf = np.zeros((P, max(col, 1) * mw, WD), np.float32)
                np.add.at(mbuf, (pp, cc * 2 + par_e, off), rec)
                self.masks[c] = mbuf.astype(F8)
                continue
            order = np.argsort(w_e * nb + b_e, kind="stable")
            rows_s, dl_s, w_s, b_s = rows[order], dl[order], w_e[order], b_e[order]
            rec_s = recip_dst_local[c][dl_s].astype(np.float32)
            rib_s = (rows_s % self.bank_rows).astype(np.int64)
            # rank within each (w, b) run
            key = w_s * nb + b_s
            # j = index within cell
            cellcnt = np.bincount(key, minlength=self.n_win * nb)
            starts = np.zeros(self.n_win * nb + 1, np.int64)
            np.cumsum(cellcnt, out=starts[1:])
            j = np.arange(len(key)) - starts[key]
            # idx slab (flat over ops)
            flat = np.zeros(max(self.idx_width, 1) * 16, np.int16)
            iobase = np.array([cell_base.get((w, b), -1) * 16
                               for w in range(self.n_win) for b in range(nb)]
                              ).reshape(self.n_win, nb)
            pos = iobase[w_s, b_s] + j
            flat[pos] = rib_s.astype(np.int16)
            w16 = flat.reshape(-1, 16).T       # [16, width]
            self.idx16[c] = np.tile(w16, (8, 1))
            # masks
            gcol = np.array([lcb_map.get((w, b), 0)
                             for w in range(self.n_win) for b in range(nb)]
                            ).reshape(self.n_win, nb)
            cc = gcol[w_s, b_s] + j // P
            pp = j % P
            off = dl_s - w_s * WD
            self.masks[c][pp, cc, off] = rec_s.astype(F8)


def _pair_srcs(srcs, dstls):
    """Per-core greedy pairing of gather sources by co-window occurrence.
    Returns per-core (pair_id per edge, parity per edge, pair row list)."""
    out = []
    for c in range(C):
        s, dl = srcs[c], dstls[c]
        w = dl // WD
        partner = {}
        for wi in range(int(w.max()) + 1):
            ss = np.unique(s[w == wi])
            free = [int(x) for x in ss if int(x) not in partner]
            for a, b in zip(free[0::2], free[1::2]):
                partner[a] = b
                partner[b] = a
        uniq = np.unique(s)
        left = [int(x) for x in uniq if int(x) not in partner]
        for a, b in zip(left[0::2], left[1::2]):
            partner[a] = b
            partner[b] = a
        if len(left) % 2:
            partner[left[-1]] = -1
        pair_rows = []
        pair_of, parity_of = {}, {}
        for x in uniq:
            x = int(x)
            if x in pair_of:
                continue
            p = partner[x]
            k = len(pair_rows)
            if p == -1:
                pair_rows.append((x, x))
                pair_of[x] = k
                parity_of[x] = 0
            else:
                pair_rows.append((x, p))
                pair_of[x] = k
                parity_of[x] = 0
                pair_of[p] = k
                parity_of[p] = 1
        ids = np.array([pair_of[int(x)] for x in s], np.int64)
        par = np.array([parity_of[int(x)] for x in s], np.int64)
        out.append((ids, par, pair_rows))
    return out


def _balance_perm(deg, n_nodes, can):
    """Permutation node -> new global id, dealing nodes into (core, window)
    cells so per-cell degree sums are balanced (pool is perm-invariant)."""
    import heapq
    n_win = ((can + P - 1) // P + WIN - 1) // WIN
    caps, base = [], []
    for c in range(C):
        for w in range(n_win):
            cap = min(WD, can - w * WD)
            caps.append(cap)
            base.append(c * can + w * WD)
    order = np.argsort(-deg, kind="stable")
    heap = [(0.0, i) for i in range(len(caps))]
    heapq.heapify(heap)
    fill = np.zeros(len(caps), np.int64)
    perm = np.empty(n_nodes, np.int64)
    for nd in order:
        while True:
            s, i = heapq.heappop(heap)
            if fill[i] < caps[i]:
                break
        perm[nd] = base[i] + fill[i]
        fill[i] += 1
        if fill[i] < caps[i]:
            heapq.heappush(heap, (s + deg[nd], i))
    return perm


def _prep(inputs):
    f = lambda k: np.asarray(inputs[k], np.float32)
    x_author, x_paper = f("x_author"), f("x_paper")
    ws, wd = (np.asarray(inputs["ei_writes_src"], np.int64),
              np.asarray(inputs["ei_writes_dst"], np.int64))
    bs, bd = (np.asarray(inputs["ei_wb_src"], np.int64),
              np.asarray(inputs["ei_wb_dst"], np.int64))

    # relabel nodes so per-(core, window) dst-degree sums are balanced
    pa_perm = _balance_perm(np.bincount(wd, minlength=NP_), NP_, P_CAN)
    au_perm = _balance_perm(np.bincount(bd, minlength=NA), NA, A_CAN)
    inv_pa = np.empty(NP_, np.int64)
    inv_pa[pa_perm] = np.arange(NP_)
    inv_au = np.empty(NA, np.int64)
    inv_au[au_perm] = np.arange(NA)
    x_paper = x_paper[inv_pa]
    x_author = x_author[inv_au]
    wd, bs = pa_perm[wd], pa_perm[bs]
    ws, bd = au_perm[ws], au_perm[bd]

    cnt_p = np.bincount(wd, minlength=NP_).astype(np.float32)
    cnt_a = np.bincount(bd, minlength=NA).astype(np.float32)
    recip_p = 1.0 / np.maximum(cnt_p, 1.0)
    recip_a = 1.0 / np.maximum(cnt_a, 1.0)

    # split edges by dst owner
    def split(src, dst, dst_can):
        srcs, dstls = [], []
        for c in range(C):
            m = (dst // dst_can) == c
            srcs.append(src[m])
            dstls.append((dst[m] % dst_can).astype(np.int64))
        return srcs, dstls

    w_src, w_dstl = split(ws, wd, P_CAN)     # writes: dst papers
    b_src, b_dstl = split(bs, bd, A_CAN)     # wb: dst authors

    # L1 compact PAIR tables: two co-window srcs per 256B fp8 row, so one
    # gather descriptor serves up to two edges
    pairsW = _pair_srcs(w_src, w_dstl)       # authors referenced per core
    pairsB = _pair_srcs(b_src, b_dstl)       # papers referenced per core
    rowsW = ((max(len(p[2]) for p in pairsW) + P - 1) // P) * P
    rowsB = ((max(len(p[2]) for p in pairsB) + P - 1) // P) * P
    assert rowsW <= 32768 and rowsB <= 32768
    xa_cmp = np.zeros((C, rowsW, 2 * IN), F8)
    xp_cmp = np.zeros((C, rowsB, 2 * IN), F8)
    for c in range(C):
        pr = np.array(pairsW[c][2], np.int64)
        xa_cmp[c, :len(pr), :IN] = x_author[pr[:, 0]].astype(F8)
        xa_cmp[c, :len(pr), IN:] = x_author[pr[:, 1]].astype(F8)
        pr = np.array(pairsB[c][2], np.int64)
        xp_cmp[c, :len(pr), :IN] = x_paper[pr[:, 0]].astype(F8)
        xp_cmp[c, :len(pr), IN:] = x_paper[pr[:, 1]].astype(F8)

    recip_p_loc = [recip_p[c * P_CAN:(c + 1) * P_CAN] for c in range(C)]
    recip_a_loc = [recip_a[c * A_CAN:(c + 1) * A_CAN] for c in range(C)]

    # AG row mapping for L2 tables
    agW = [(s // A_CAN) * A_PAD + (s % A_CAN) for s in w_src]
    agB = [(s // P_CAN) * P_PAD + (s % P_CAN) for s in b_src]
    rels = dict(
        W1=RelLayer([p[0] for p in pairsW], None, w_dstl, P_CAN, P_PAD,
                    recip_p_loc, rowsW, parity=[p[1] for p in pairsW]),
        B1=RelLayer([p[0] for p in pairsB], None, b_dstl, A_CAN, A_PAD,
                    recip_a_loc, rowsB, parity=[p[1] for p in pairsB]),
        W2=RelLayer(agW, None, w_dstl, P_CAN, P_PAD, recip_p_loc, NA_AG),
        B2=RelLayer(agB, None, b_dstl, A_CAN, A_PAD, recip_a_loc, NP_AG),
    )

    # fp16 local chunks (root/skip transposes)
    xa_chunk = np.zeros((C, A_PAD, IN), np.float16)
    xp_chunk = np.zeros((C, P_PAD, IN), np.float16)
    for c in range(C):
        xa_chunk[c, :A_CAN] = x_author[c * A_CAN:(c + 1) * A_CAN]
        xp_chunk[c, :P_CAN] = x_paper[c * P_CAN:(c + 1) * P_CAN]

    # weight slab: 14 x [128, 256] fp16 (transposed: [in, out])
    wT = lambda k: f(k).T.astype(np.float16)
    slabs = [wT("c1w_Wl"), wT("c1w_Wr"), wT("c1b_Wl"), wT("c1b_Wr")]
    for k in ("c2w_Wl", "c2w_Wr", "c2b_Wl", "c2b_Wr"):
        w2 = wT(k)
        slabs += [w2[:128], w2[128:]]
    slabs += [wT("skipA_W"), wT("skipP_W")]
    wslab = np.concatenate(slabs, axis=0)          # [14*128, 256]

    pool_ones = np.zeros((P, 3), np.float16)
    pool_ones[:, 0] = 1.0
    pool_ones[:P_CAN - (P_PAD // P - 1) * P, 1] = 1.0   # last paper tile mask
    pool_ones[:A_CAN - (A_PAD // P - 1) * P, 2] = 1.0   # last author tile mask

    bias_nz = {k: bool(np.any(f(k))) for k in
               ("c1w_bl", "c1b_bl", "skipA_b", "skipP_b")}
    bias_arr = {k: np.broadcast_to(f(k2), (P, H)).astype(np.float32).copy()
                for k, k2 in (("bias_p1", "c1w_bl"), ("bias_a1", "c1b_bl"),
                              ("bias_p2", "skipP_b"), ("bias_a2", "skipA_b"))}

    in_maps = []
    for c in range(C):
        m = dict(
            xa_cmp=xa_cmp[c], xp_cmp=xp_cmp[c],
            xa_chunk=xa_chunk[c], xp_chunk=xp_chunk[c],
            wslab=wslab, pool_ones=pool_ones,
        )
        for nm, rl in rels.items():
            m["idx_" + nm] = rl.idx16[c]
            m["mask_" + nm] = rl.masks[c].reshape(P, -1)
        for k, arr in bias_arr.items():
            m[k] = arr
        in_maps.append(m)
    return rels, in_maps, bias_nz


def _build(rels, bias_nz, debug=False):
    nc = bacc.Bacc("TRN2", target_bir_lowering=False, debug=False,
                   num_devices=C)
    f16, f32, i16, f8 = dt.float16, dt.float32, dt.int16, dt.float8e4
    ein = lambda n, s, d: nc.dram_tensor(n, s, d, kind="ExternalInput")

    xa_cmp = ein("xa_cmp", [rels["W1"].table_rows, 2 * IN], f8)
    xp_cmp = ein("xp_cmp", [rels["B1"].table_rows, 2 * IN], f8)
    xa_chunk = ein("xa_chunk", [A_PAD, IN], f16)
    xp_chunk = ein("xp_chunk", [P_PAD, IN], f16)
    wslab = ein("wslab", [14 * P, H], f16)
    pool_in = ein("pool_ones", [P, 3], f16)
    idx_h, mask_h = {}, {}
    for nm, rl in rels.items():
        mw = 2 if rl.paired else 1
        idx_h[nm] = ein("idx_" + nm, [P, max(rl.idx_width, 1)], i16)
        mask_h[nm] = ein("mask_" + nm, [P, max(rl.total_cols, 1) * mw * WD], f8)
    bias_in = {k: ein(k, [P, H], f32)
               for k in ("bias_p1", "bias_a1", "bias_p2", "bias_a2")}

    out_pool = nc.dram_tensor("out_pool", [1, 2 * H], f32,
                              kind="ExternalOutput")
    if debug:
        dbg_h1a = nc.dram_tensor("dbg_h1a", [A_PAD, H], f16,
                                 kind="ExternalOutput")
        dbg_h1p = nc.dram_tensor("dbg_h1p", [P_PAD, H], f16,
                                 kind="ExternalOutput")

    W = {k: i for i, k in enumerate(
        ["c1w_Wl", "c1w_Wr", "c1b_Wl", "c1b_Wr",
         "c2w_Wl0", "c2w_Wl1", "c2w_Wr0", "c2w_Wr1",
         "c2b_Wl0", "c2b_Wl1", "c2b_Wr0", "c2b_Wr1",
         "skipA_W", "skipP_W"])}
    relu_f = mybir.ActivationFunctionType.Relu
    rg = [list(range(C))]
    MAXW = max(rl.max_wcols for rl in rels.values())
    MAXM = max(rl.max_wcols * (2 if rl.paired else 1) for rl in rels.values())

    with tile.TileContext(nc) as tc:
        with tc.tile_pool(name="persist", bufs=1) as pp, \
             tc.tile_pool(name="dram", bufs=1, space="DRAM") as dp, \
             tc.tile_pool(name="work", bufs=3) as wk, \
             tc.tile_pool(name="msgs", bufs=3) as mp, \
             tc.tile_pool(name="maskp", bufs=2) as mk, \
             tc.tile_pool(name="psA", bufs=4, space="PSUM") as psA, \
             tc.tile_pool(name="psL", bufs=2, space="PSUM") as psL, \
             tc.tile_pool(name="psP", bufs=1, space="PSUM") as psP:

            # ---------------- persistent loads (idx first: gathers need it)
            idx_t = {}
            for nm in ("B1", "W1", "W2", "B2"):
                rl = rels[nm]
                t = pp.tile([P, max(rl.idx_width, 1)], i16, name="idx" + nm)
                nc.sync.dma_start(out=t[:], in_=idx_h[nm][:])
                idx_t[nm] = t
            wt = pp.tile([P, 14, H], f16, name="wt", tag="wt")
            nc.sync.dma_start(out=wt[:],
                              in_=wslab[:].rearrange("(s p) d -> p s d", p=P))
            pool_t = pp.tile([P, 3], f16, name="pool_t", tag="pool_t")
            nc.sync.dma_start(out=pool_t[:], in_=pool_in[:])
            bias_t = {}
            for k, nz in (("bias_p1", bias_nz["c1w_bl"]),
                          ("bias_a1", bias_nz["c1b_bl"]),
                          ("bias_p2", bias_nz["skipP_b"]),
                          ("bias_a2", bias_nz["skipA_b"])):
                if nz:
                    t = pp.tile([P, H], f32, name=k + "_t")
                    nc.sync.dma_start(out=t[:], in_=bias_in[k][:])
                    bias_t[k] = t

            xaT = pp.tile([P, A_PAD], f16, name="xaT", tag="xaT")
            nc.sync.dma_start_transpose(out=xaT[:], in_=xa_chunk[:])
            xpT = pp.tile([P, P_PAD], f16, name="xpT", tag="xpT")
            nc.sync.dma_start_transpose(out=xpT[:], in_=xp_chunk[:])

            # h1 tables: fp8 local shard -> Shared-output AllGather table
            # (fp16 local shard feeds the DMA transposes for L2 root terms)
            h1a_sh = dp.tile([NA_AG, H], f8, name="h1a_sh", tag="h1a_sh",
                             addr_space="Shared")
            h1p_sh = dp.tile([NP_AG, H], f8, name="h1p_sh", tag="h1p_sh",
                             addr_space="Shared")
            h1a_l8 = dp.tile([A_PAD, H], f8, name="h1a_l8", tag="h1a_l8")
            h1p_l8 = dp.tile([P_PAD, H], f8, name="h1p_l8", tag="h1p_l8")
            h1a_loc = dp.tile([A_PAD, H], f16, name="h1a_loc", tag="h1a_loc")
            h1p_loc = dp.tile([P_PAD, H], f16, name="h1p_loc", tag="h1p_loc")

            def conv(nm, table, elem, Wl, Wr, rootT, skipW, skipT, bias,
                     h_l8, h_loc, pool_ps, pool_last_col):
                rl = rels[nm]
                nslice = elem // P
                it = idx_t[nm]
                fetch = 2 * elem if rl.paired else elem
                mw = 2 if rl.paired else 1
                gathers = []
                for w in range(rl.n_win):
                    wc = int(rl.wcols[w])
                    cb = int(rl.col_base[w])
                    aggT = []
                    if wc:
                        msgs = mp.tile([P, MAXW, 256], f8, tag="msgs")
                        for (b, ioff, nidx, lcb) in rl.ops[w]:
                            b0 = b * rl.bank_rows
                            b1 = min(b0 + rl.bank_rows, rl.table_rows)
                            gathers.append(nc.gpsimd.dma_gather(
                                msgs[:, lcb:lcb + nidx // P, :fetch],
                                table[b0:b1, :],
                                it[:, ioff:ioff + nidx // 16],
                                nidx, nidx, fetch, single_packet=False))
                        mask_t = mk.tile([P, MAXM * WD], f8, tag="mask")
                        nc.scalar.dma_start(
                            out=mask_t[:, :wc * mw * WD],
                            in_=mask_h[nm][:, cb * mw * WD:
                                           (cb + wc) * mw * WD])
                        aggs = []
                        for s in range(nslice):
                            aggs.append(psA.tile([P, WD], f32, tag="agg",
                                                 name="agg", space="PSUM"))
                        for i in range(wc):
                            for h in range(mw):
                                for s in range(nslice):
                                    nc.tensor.matmul(
                                        out=aggs[s][:],
                                        lhsT=msgs[:, i:i + 1,
                                                  (h * nslice + s) * P:
                                                  (h * nslice + s + 1) * P],
                                        rhs=mask_t[:, (mw * i + h) * WD:
                                                   (mw * i + h + 1) * WD],
                                        start=(i == 0 and h == 0),
                                        stop=(i == wc - 1 and h == mw - 1))
                        for s in range(nslice):
                            a = wk.tile([P, WD], f16, tag="aggT")
                            nc.scalar.copy(out=a[:], in_=aggs[s][:])
                            aggT.append(a)
                    for tl in range(min(WIN, rl.n_tiles - w * WIN)):
                        t = w * WIN + tl
                        lin = psL.tile([P, H], f32, tag="lin", space="PSUM")
                        first = True
                        if wc:
                            for s in range(nslice):
                                nc.tensor.matmul(
                                    out=lin[:],
                                    lhsT=aggT[s][:, tl * P:(tl + 1) * P],
                                    rhs=wt[:, Wl[s]:Wl[s] + 1, :],
                                    start=first, stop=False)
                                first = False
                        for s in range(nslice):
                            nc.tensor.matmul(
                                out=lin[:],
                                lhsT=rootT[s][:, t * P:(t + 1) * P],
                                rhs=wt[:, Wr[s]:Wr[s] + 1, :],
                                start=first,
                                stop=(skipW is None and s == nslice - 1))
                            first = False
                        if skipW is not None:
                            nc.tensor.matmul(
                                out=lin[:], lhsT=skipT[:, t * P:(t + 1) * P],
                                rhs=wt[:, skipW:skipW + 1, :],
                                start=False, stop=True)
                        h16 = wk.tile([P, H], f16, tag="h16")
                        if bias is None:
                            src = lin
                        else:
                            tmp = wk.tile([P, H], f32, tag="btmp")
                            nc.vector.tensor_add(out=tmp[:], in0=lin[:],
                                                 in1=bias[:])
                            src = tmp
                        nc.scalar.activation(out=h16[:], in_=src[:],
                                             func=relu_f)
                        if h_l8 is not None:
                            h8 = wk.tile([P, H], f8, tag="h8")
                            nc.scalar.activation(out=h8[:], in_=src[:],
                                                 func=relu_f)
                            nc.scalar.dma_start(
                                out=h_l8[t * P:(t + 1) * P, :], in_=h8[:])
                            nc.scalar.dma_start(
                                out=h_loc[t * P:(t + 1) * P, :], in_=h16[:])
                        if pool_ps is not None:
                            oc = pool_last_col if t == rl.n_tiles - 1 else 0
                            nc.tensor.matmul(
                                out=pool_ps[:], lhsT=pool_t[:, oc:oc + 1],
                                rhs=h16[:], start=(t == 0),
                                stop=(t == rl.n_tiles - 1),
                                skip_group_check=True)
                return gathers

            # -------- layer 1: authors (wb: src papers -> dst authors)
            conv("B1", xp_cmp, IN, [W["c1b_Wl"]], [W["c1b_Wr"]], [xaT],
                 None, None, bias_t.get("bias_a1"), h1a_l8, h1a_loc, None, 0)
            h1aT = []
            for s in range(2):
                t = pp.tile([P, A_PAD], f16, name=f"h1aT{s}", tag=f"h1aT{s}")
                nc.sync.dma_start_transpose(
                    out=t[:], in_=h1a_loc[:, s * P:(s + 1) * P])
                h1aT.append(t)
            nc.gpsimd.collective_compute(
                "AllGather", mybir.AluOpType.bypass, replica_groups=rg,
                ins=[h1a_l8.opt()], outs=[h1a_sh.opt()])

            # -------- layer 1: papers (writes: src authors -> dst papers)
            conv("W1", xa_cmp, IN, [W["c1w_Wl"]], [W["c1w_Wr"]], [xpT],
                 None, None, bias_t.get("bias_p1"), h1p_l8, h1p_loc, None, 0)
            h1pT = []
            for s in range(2):
                t = pp.tile([P, P_PAD], f16, name=f"h1pT{s}", tag=f"h1pT{s}")
                nc.sync.dma_start_transpose(
                    out=t[:], in_=h1p_loc[:, s * P:(s + 1) * P])
                h1pT.append(t)

            # -------- layer 2: papers (gathers h1a from shared table)
            pool_p = psP.tile([1, H], f32, name="pool_p", tag="pool_p",
                              space="PSUM")
            pool_a = psP.tile([1, H], f32, name="pool_a", tag="pool_a",
                              space="PSUM")
            gW2 = conv("W2", h1a_sh, H, [W["c2w_Wl0"], W["c2w_Wl1"]],
                       [W["c2w_Wr0"], W["c2w_Wr1"]], h1pT, W["skipP_W"], xpT,
                       bias_t.get("bias_p2"), None, None, pool_p, 1)

            # AG(h1p): L2-papers does not consume it; pin it behind the last
            # L2-papers gather so the scheduler cannot hoist its inline wait
            # into the middle of the gather stream
            ccP = nc.gpsimd.collective_compute(
                "AllGather", mybir.AluOpType.bypass, replica_groups=rg,
                ins=[h1p_l8.opt()], outs=[h1p_sh.opt()])
            _add_dep_helper(ccP.ins, gW2[-1].ins,
                            reason="keep AG(h1p) after L2-papers gathers")

            # -------- layer 2: authors
            conv("B2", h1p_sh, H, [W["c2b_Wl0"], W["c2b_Wl1"]],
                 [W["c2b_Wr0"], W["c2b_Wr1"]], h1aT, W["skipA_W"], xaT,
                 bias_t.get("bias_a2"), None, None, pool_a, 2)

            pool_sb = wk.tile([1, 2 * H], f32, tag="poolout")
            nc.vector.tensor_copy(out=pool_sb[:, 0:H], in_=pool_a[:])
            nc.vector.tensor_copy(out=pool_sb[:, H:2 * H], in_=pool_p[:])
            nc.sync.dma_start(out=out_pool[:], in_=pool_sb[:])

            if debug:
                nc.sync.dma_start(out=dbg_h1a[:], in_=h1a_loc[:])
                nc.sync.dma_start(out=dbg_h1p[:], in_=h1p_loc[:])

    nc.compile()
    return nc


def kernel(**inputs):
    debug = bool(int(os.environ.get("GNN_DEBUG", "0")))
    trace = bool(int(os.environ.get("GNN_TRACE", "0")))
    rels, in_maps, bias_nz = _prep(inputs)
    nc = _build(rels, bias_nz, debug=debug)
    res = bass_utils.run_bass_kernel_spmd(
        nc, in_maps, core_ids=list(range(C)), trace=trace)
    kernel.last_results = res

    pools = np.stack([res.results[c]["out_pool"] for c in range(C)])
    sum_a = pools[:, 0, :H].astype(np.float64).sum(axis=0)
    sum_p = pools[:, 0, H:].astype(np.float64).sum(axis=0)
    pooled = np.concatenate([sum_a / NA, sum_p / NP_])[None, :]
    W1 = np.asarray(inputs["cls_W1"], np.float64)
    b1 = np.asarray(inputs["cls_b1"], np.float64)
    W2 = np.asarray(inputs["cls_W2"], np.float64)
    b2 = np.asarray(inputs["cls_b2"], np.float64)
    h = np.maximum(pooled @ W1.T + b1, 0.0)
    out = h @ W2.T + b2
    return out.astype(np.float32)


# revision 41
# speedup vs baseline: 1.1437x; 1.0709x over previous
"""Trainium2 Bass kernel for the GSAT HeteroGNN problem (8 NeuronCores).

Self-contained: hardcodes shapes/sharding; only imports the concourse
toolchain.

Strategy (dst-node sharding, SPMD over 8 cores):
  - papers split into 8 canonical chunks of 12500 (padded 12544 = 98 tiles),
    authors 8 x 6250 (padded 6272 = 49 tiles).
  - edges live on their dst's owner core, laid out host-side into 128-slot
    columns per (4-tile window, src-bank); dma_gather (int16 idx) fetches
    fp8 source rows as [128, cols, feat].
  - segment-mean via host-precomputed fp8 masks streamed by DMA:
    mask[slot, dst_in_window] = 1/deg(dst); TensorE accumulates
    aggT[feat, 512] in PSUM per window (no on-device mask building).
  - L1 gathers read per-core COMPACT fp8 tables (only the <=32k rows this
    core references -> single bank, minimal column padding).
  - L1 outputs h1 are written twice: fp8 rows into a local chunk that a
    Shared-output AllGather assembles into a shared fp8 table (each rank
    contributes only its 1.6-3.2MB shard; the old Local-output AllGathers
    moved 77MB/core), and fp16 into a local chunk used for DMA transposes
    (L2 root terms).
  - L2 gathers read the shared fp8 h1 tables directly.
  - all DMA transposes are placed before any collective in program order
    (the scheduler serializes transposes with collectives).
  - global mean-pool via ones-column matmuls accumulating in PSUM; final
    2-layer MLP on host in fp64.
"""
import os
import sys

try:
    import concourse  # noqa: F401
except ImportError:  # toolchain location in the grading container
    sys.path.insert(0, "/opt/trn_rl_repo")

import numpy as np
import ml_dtypes
from concourse import bass, bacc, mybir, tile  # noqa: F401
from concourse import bass_utils
from concourse.bass import _add_dep_helper

dt = mybir.dt
F8 = ml_dtypes.float8_e4m3

# ---------------------------------------------------------------- constants
NA, NP_, E = 50000, 100000, 300000
IN, H, OUT = 128, 256, 16
C = 8                      # cores
P = 128                    # partitions
A_CAN, P_CAN = NA // C, NP_ // C              # 6250 / 12500
A_PAD = ((A_CAN + P - 1) // P) * P            # 6272
P_PAD = ((P_CAN + P - 1) // P) * P            # 12544
NA_AG, NP_AG = C * A_PAD, C * P_PAD           # 50176 / 100352
WIN = 4                    # dst tiles per PSUM window (512 dsts)
WD = WIN * P               # window width in dsts


class RelLayer:
    """Host-side layout for one (relation, layer): slot columns per
    (window, bank), uniform across cores (max-over-cores column counts),
    int16 gather indices and fp8 recip masks."""

    def __init__(self, row_of, dst_owner, dstl, n_dst_can, n_dst_pad,
                 recip_dst_local, table_rows, parity=None):
        # parity: per-core per-edge 0/1 within its pair-row (paired mode);
        # row_of then holds PAIR ids and slots dedupe by (window, pair)
        # row_of: [C] list of per-edge row ids (into this layer's table)
        # dst_owner/dstl: per-edge owner core and local dst id (global arrays
        # already split: row_of[c] aligned with dstl[c])
        self.n_tiles = n_dst_pad // P
        self.n_win = (self.n_tiles + WIN - 1) // WIN
        nb = (table_rows + 32767) // 32768
        self.n_banks = nb
        self.bank_rows = (table_rows + nb - 1) // nb
        self.table_rows = table_rows

        self.paired = parity is not None
        # per-core per-cell counts -> uniform ncols
        ncols = np.zeros((self.n_win, nb), np.int64)
        per_core = []
        for c in range(C):
            rows, dl = row_of[c], dstl[c]
            par = parity[c] if self.paired else np.zeros(len(rows), np.int64)
            w = dl // WD
            b = rows // self.bank_rows
            if self.paired:
                # one slot per distinct (window, pair-row)
                key = w * self.table_rows + rows
                uk, inv = np.unique(key, return_inverse=True)
                cnt = np.zeros((self.n_win, nb), np.int64)
                np.add.at(cnt, ((uk // self.table_rows),
                                (uk % self.table_rows) // self.bank_rows), 1)
            else:
                inv = None
                cnt = np.zeros((self.n_win, nb), np.int64)
                np.add.at(cnt, (w, b), 1)
            ncols = np.maximum(ncols, (cnt + P - 1) // P)
            per_core.append((rows, dl, w, b, par, inv))
        self.ncols = ncols

        # global column layout: window-major, bank-minor
        self.col_base = np.zeros(self.n_win + 1, np.int64)
        self.ops = []              # per window: list of (bank, ioff, nidx, lcb)
        ioff = 0
        col = 0
        for w in range(self.n_win):
            self.col_base[w] = col
            wops = []
            lcb = 0
            for b in range(nb):
                nco = int(ncols[w, b])
                if nco:
                    wops.append((b, ioff, nco * P, lcb))
                    ioff += nco * P // 16
                    lcb += nco
                    col += nco
            self.ops.append(wops)
        self.col_base[self.n_win] = col
        self.total_cols = col
        self.idx_width = ioff
        self.wcols = np.diff(self.col_base).astype(np.int64)
        self.max_wcols = int(self.wcols.max()) if col else 0
        self.total_idx = col * P

        # per-core idx + masks
        self.idx16 = np.zeros((C, P, max(self.idx_width, 1)), np.int16)
        self.masks = np.zeros((C, P, max(col, 1), WD), F8)  # repl. if paired
        cell_base = {}
        lcb_map = {}
        for w in range(self.n_win):
            for (b, io, nidx, lcb) in self.ops[w]:
                cell_base[(w, b)] = io
                lcb_map[(w, b)] = self.col_base[w] + lcb
        mw = 2 if self.paired else 1
        self.masks = np.zeros((C, P, max(col, 1) * mw, WD), F8)
        for c in range(C):
            rows, dl, w_e, b_e, par_e, inv = per_core[c]
            if self.paired:
                # slot per distinct (w, pair): compute slot ranks per cell
                key_sl = w_e * self.table_rows + rows
                uk = np.unique(key_sl)
                sw = uk // self.table_rows
                srow = uk % self.table_rows
                sb = srow // self.bank_rows
                cellkey = sw * nb + sb
                cellcnt = np.bincount(cellkey, minlength=self.n_win * nb)
                starts = np.zeros(self.n_win * nb + 1, np.int64)
                np.cumsum(cellcnt, out=starts[1:])
                jslot = np.arange(len(uk)) - starts[cellkey]
                # idx slab: one idx per slot
                flat = np.zeros(max(self.idx_width, 1) * 16, np.int16)
                iobase = np.array([cell_base.get((w, b), -1) * 16
                                   for w in range(self.n_win)
                                   for b in range(nb)]).reshape(self.n_win, nb)
                flat[iobase[sw, sb] + jslot] = (srow % self.bank_rows
                                                ).astype(np.int16)
                w16 = flat.reshape(-1, 16).T
                self.idx16[c] = np.tile(w16, (8, 1))
                # masks: edges land at their slot, plane = parity
                gcol = np.array([lcb_map.get((w, b), 0)
                                 for w in range(self.n_win)
                                 for b in range(nb)]).reshape(self.n_win, nb)
                slot_of_edge = np.searchsorted(uk, key_sl)
                je = jslot[slot_of_edge]
                cc = gcol[sw[slot_of_edge], sb[slot_of_edge]] + je // P
                pp = je % P
                off = dl - w_e * WD
                rec = recip_dst_local[c][dl].astype(np.float32)
                mbuf = np.zeros((P, max(col, 1) * mw, WD), np.float32)
                np.add.at(mbuf, (pp, cc * 2 + par_e, off), rec)
                self.masks[c] = mbuf.astype(F8)
                continue
            order = np.argsort(w_e * nb + b_e, kind="stable")
            rows_s, dl_s, w_s, b_s = rows[order], dl[order], w_e[order], b_e[order]
            rec_s = recip_dst_local[c][dl_s].astype(np.float32)
            rib_s = (rows_s % self.bank_rows).astype(np.int64)
            # rank within each (w, b) run
            key = w_s * nb + b_s
            # j = index within cell
            cellcnt = np.bincount(key, minlength=self.n_win * nb)
            starts = np.zeros(self.n_win * nb + 1, np.int64)
            np.cumsum(cellcnt, out=starts[1:])
            j = np.arange(len(key)) - starts[key]
            # idx slab (flat over ops)
            flat = np.zeros(max(self.idx_width, 1) * 16, np.int16)
            iobase = np.array([cell_base.get((w, b), -1) * 16
                               for w in range(self.n_win) for b in range(nb)]
                              ).reshape(self.n_win, nb)
            pos = iobase[w_s, b_s] + j
            flat[pos] = rib_s.astype(np.int16)
            w16 = flat.reshape(-1, 16).T       # [16, width]
            self.idx16[c] = np.tile(w16, (8, 1))
            # masks
            gcol = np.array([lcb_map.get((w, b), 0)
                             for w in range(self.n_win) for b in range(nb)]
                            ).reshape(self.n_win, nb)
            cc = gcol[w_s, b_s] + j // P
            pp = j % P
            off = dl_s - w_s * WD
            self.masks[c][pp, cc, off] = rec_s.astype(F8)


def _pair_srcs(srcs, dstls):
    """Per-core greedy pairing of gather sources by co-window occurrence.
    Returns per-core (pair_id per edge, parity per edge, pair row list)."""
    out = []
    for c in range(C):
        s, dl = srcs[c], dstls[c]
        w = dl // WD
        partner = {}
        for wi in range(int(w.max()) + 1):
            ss = np.unique(s[w == wi])
            free = [int(x) for x in ss if int(x) not in partner]
            for a, b in zip(free[0::2], free[1::2]):
                partner[a] = b
                partner[b] = a
        uniq = np.unique(s)
        left = [int(x) for x in uniq if int(x) not in partner]
        for a, b in zip(left[0::2], left[1::2]):
            partner[a] = b
            partner[b] = a
        if len(left) % 2:
            partner[left[-1]] = -1
        pair_rows = []
        pair_of, parity_of = {}, {}
        for x in uniq:
            x = int(x)
            if x in pair_of:
                continue
            p = partner[x]
            k = len(pair_rows)
            if p == -1:
                pair_rows.append((x, x))
                pair_of[x] = k
                parity_of[x] = 0
            else:
                pair_rows.append((x, p))
                pair_of[x] = k
                parity_of[x] = 0
                pair_of[p] = k
                parity_of[p] = 1
        ids = np.array([pair_of[int(x)] for x in s], np.int64)
        par = np.array([parity_of[int(x)] for x in s], np.int64)
        out.append((ids, par, pair_rows))
    return out


def _balance_perm(deg, n_nodes, can):
    """Permutation node -> new global id, dealing nodes into (core, window)
    cells so per-cell degree sums are balanced (pool is perm-invariant)."""
    import heapq
    n_win = ((can + P - 1) // P + WIN - 1) // WIN
    caps, base = [], []
    for c in range(C):
        for w in range(n_win):
            cap = min(WD, can - w * WD)
            caps.append(cap)
            base.append(c * can + w * WD)
    order = np.argsort(-deg, kind="stable")
    heap = [(0.0, i) for i in range(len(caps))]
    heapq.heapify(heap)
    fill = np.zeros(len(caps), np.int64)
    perm = np.empty(n_nodes, np.int64)
    for nd in order:
        while True:
            s, i = heapq.heappop(heap)
            if fill[i] < caps[i]:
                break
        perm[nd] = base[i] + fill[i]
        fill[i] += 1
        if fill[i] < caps[i]:
            heapq.heappush(heap, (s + deg[nd], i))
    return perm


def _prep(inputs):
    f = lambda k: np.asarray(inputs[k], np.float32)
    x_author, x_paper = f("x_author"), f("x_paper")
    ws, wd = (np.asarray(inputs["ei_writes_src"], np.int64),
              np.asarray(inputs["ei_writes_dst"], np.int64))
    bs, bd = (np.asarray(inputs["ei_wb_src"], np.int64),
              np.asarray(inputs["ei_wb_dst"], np.int64))

    # relabel nodes so per-(core, window) dst-degree sums are balanced
    pa_perm = _balance_perm(np.bincount(wd, minlength=NP_), NP_, P_CAN)
    au_perm = _balance_perm(np.bincount(bd, minlength=NA), NA, A_CAN)
    inv_pa = np.empty(NP_, np.int64)
    inv_pa[pa_perm] = np.arange(NP_)
    inv_au = np.empty(NA, np.int64)
    inv_au[au_perm] = np.arange(NA)
    x_paper = x_paper[inv_pa]
    x_author = x_author[inv_au]
    wd, bs = pa_perm[wd], pa_perm[bs]
    ws, bd = au_perm[ws], au_perm[bd]

    cnt_p = np.bincount(wd, minlength=NP_).astype(np.float32)
    cnt_a = np.bincount(bd, minlength=NA).astype(np.float32)
    recip_p = 1.0 / np.maximum(cnt_p, 1.0)
    recip_a = 1.0 / np.maximum(cnt_a, 1.0)

    # split edges by dst owner
    def split(src, dst, dst_can):
        srcs, dstls = [], []
        for c in range(C):
            m = (dst // dst_can) == c
            srcs.append(src[m])
            dstls.append((dst[m] % dst_can).astype(np.int64))
        return srcs, dstls

    w_src, w_dstl = split(ws, wd, P_CAN)     # writes: dst papers
    b_src, b_dstl = split(bs, bd, A_CAN)     # wb: dst authors

    # L1 compact PAIR tables: two co-window srcs per 256B fp8 row, so one
    # gather descriptor serves up to two edges
    pairsW = _pair_srcs(w_src, w_dstl)       # authors referenced per core
    pairsB = _pair_srcs(b_src, b_dstl)       # papers referenced per core
    rowsW = ((max(len(p[2]) for p in pairsW) + P - 1) // P) * P
    rowsB = ((max(len(p[2]) for p in pairsB) + P - 1) // P) * P
    assert rowsW <= 32768 and rowsB <= 32768
    xa_cmp = np.zeros((C, rowsW, 2 * IN), F8)
    xp_cmp = np.zeros((C, rowsB, 2 * IN), F8)
    for c in range(C):
        pr = np.array(pairsW[c][2], np.int64)
        xa_cmp[c, :len(pr), :IN] = x_author[pr[:, 0]].astype(F8)
        xa_cmp[c, :len(pr), IN:] = x_author[pr[:, 1]].astype(F8)
        pr = np.array(pairsB[c][2], np.int64)
        xp_cmp[c, :len(pr), :IN] = x_paper[pr[:, 0]].astype(F8)
        xp_cmp[c, :len(pr), IN:] = x_paper[pr[:, 1]].astype(F8)

    recip_p_loc = [recip_p[c * P_CAN:(c + 1) * P_CAN] for c in range(C)]
    recip_a_loc = [recip_a[c * A_CAN:(c + 1) * A_CAN] for c in range(C)]

    # AG row mapping for L2 tables
    agW = [(s // A_CAN) * A_PAD + (s % A_CAN) for s in w_src]
    agB = [(s // P_CAN) * P_PAD + (s % P_CAN) for s in b_src]
    rels = dict(
        W1=RelLayer([p[0] for p in pairsW], None, w_dstl, P_CAN, P_PAD,
                    recip_p_loc, rowsW, parity=[p[1] for p in pairsW]),
        B1=RelLayer([p[0] for p in pairsB], None, b_dstl, A_CAN, A_PAD,
                    recip_a_loc, rowsB, parity=[p[1] for p in pairsB]),
        W2=RelLayer(agW, None, w_dstl, P_CAN, P_PAD, recip_p_loc, NA_AG),
        B2=RelLayer(agB, None, b_dstl, A_CAN, A_PAD, recip_a_loc, NP_AG),
    )

    # fp16 local chunks (root/skip transposes)
    xa_chunk = np.zeros((C, A_PAD, IN), np.float16)
    xp_chunk = np.zeros((C, P_PAD, IN), np.float16)
    for c in range(C):
        xa_chunk[c, :A_CAN] = x_author[c * A_CAN:(c + 1) * A_CAN]
        xp_chunk[c, :P_CAN] = x_paper[c * P_CAN:(c + 1) * P_CAN]

    # weight slab: 14 x [128, 256] fp16 (transposed: [in, out])
    wT = lambda k: f(k).T.astype(np.float16)
    slabs = [wT("c1w_Wl"), wT("c1w_Wr"), wT("c1b_Wl"), wT("c1b_Wr")]
    for k in ("c2w_Wl", "c2w_Wr", "c2b_Wl", "c2b_Wr"):
        w2 = wT(k)
        slabs += [w2[:128], w2[128:]]
    slabs += [wT("skipA_W"), wT("skipP_W")]
    wslab = np.concatenate(slabs, axis=0)          # [14*128, 256]

    pool_ones = np.zeros((P, 3), np.float16)
    pool_ones[:, 0] = 1.0
    pool_ones[:P_CAN - (P_PAD // P - 1) * P, 1] = 1.0   # last paper tile mask
    pool_ones[:A_CAN - (A_PAD // P - 1) * P, 2] = 1.0   # last author tile mask

    bias_nz = {k: bool(np.any(f(k))) for k in
               ("c1w_bl", "c1b_bl", "skipA_b", "skipP_b")}
    bias_arr = {k: np.broadcast_to(f(k2), (P, H)).astype(np.float32).copy()
                for k, k2 in (("bias_p1", "c1w_bl"), ("bias_a1", "c1b_bl"),
                              ("bias_p2", "skipP_b"), ("bias_a2", "skipA_b"))}

    in_maps = []
    for c in range(C):
        m = dict(
            xa_cmp=xa_cmp[c], xp_cmp=xp_cmp[c],
            xa_chunk=xa_chunk[c], xp_chunk=xp_chunk[c],
            wslab=wslab, pool_ones=pool_ones,
        )
        for nm, rl in rels.items():
            m["idx_" + nm] = rl.idx16[c]
            m["mask_" + nm] = rl.masks[c].reshape(P, -1)
        for k, arr in bias_arr.items():
            m[k] = arr
        in_maps.append(m)
    return rels, in_maps, bias_nz


def _build(rels, bias_nz, debug=False):
    nc = bacc.Bacc("TRN2", target_bir_lowering=False, debug=False,
                   num_devices=C)
    f16, f32, i16, f8 = dt.float16, dt.float32, dt.int16, dt.float8e4
    ein = lambda n, s, d: nc.dram_tensor(n, s, d, kind="ExternalInput")

    xa_cmp = ein("xa_cmp", [rels["W1"].table_rows, 2 * IN], f8)
    xp_cmp = ein("xp_cmp", [rels["B1"].table_rows, 2 * IN], f8)
    xa_chunk = ein("xa_chunk", [A_PAD, IN], f16)
    xp_chunk = ein("xp_chunk", [P_PAD, IN], f16)
    wslab = ein("wslab", [14 * P, H], f16)
    pool_in = ein("pool_ones", [P, 3], f16)
    idx_h, mask_h = {}, {}
    for nm, rl in rels.items():
        mw = 2 if rl.paired else 1
        idx_h[nm] = ein("idx_" + nm, [P, max(rl.idx_width, 1)], i16)
        mask_h[nm] = ein("mask_" + nm, [P, max(rl.total_cols, 1) * mw * WD], f8)
    bias_in = {k: ein(k, [P, H], f32)
               for k in ("bias_p1", "bias_a1", "bias_p2", "bias_a2")}

    out_pool = nc.dram_tensor("out_pool", [1, 2 * H], f32,
                              kind="ExternalOutput")
    if debug:
        dbg_h1a = nc.dram_tensor("dbg_h1a", [A_PAD, H], f16,
                                 kind="ExternalOutput")
        dbg_h1p = nc.dram_tensor("dbg_h1p", [P_PAD, H], f16,
                                 kind="ExternalOutput")

    W = {k: i for i, k in enumerate(
        ["c1w_Wl", "c1w_Wr", "c1b_Wl", "c1b_Wr",
         "c2w_Wl0", "c2w_Wl1", "c2w_Wr0", "c2w_Wr1",
         "c2b_Wl0", "c2b_Wl1", "c2b_Wr0", "c2b_Wr1",
         "skipA_W", "skipP_W"])}
    relu_f = mybir.ActivationFunctionType.Relu
    rg = [list(range(C))]
    MAXW = max(rl.max_wcols for rl in rels.values())
    MAXM = max(rl.max_wcols * (2 if rl.paired else 1) for rl in rels.values())

    with tile.TileContext(nc) as tc:
        with tc.tile_pool(name="persist", bufs=1) as pp, \
             tc.tile_pool(name="dram", bufs=1, space="DRAM") as dp, \
             tc.tile_pool(name="work", bufs=3) as wk, \
             tc.tile_pool(name="msgs", bufs=3) as mp, \
             tc.tile_pool(name="maskp", bufs=2) as mk, \
             tc.tile_pool(name="psA", bufs=4, space="PSUM") as psA, \
             tc.tile_pool(name="psL", bufs=2, space="PSUM") as psL, \
             tc.tile_pool(name="psP", bufs=1, space="PSUM") as psP:

            # ---------------- persistent loads (idx first: gathers need it)
            idx_t = {}
            for nm in ("B1", "W1", "W2", "B2"):
                rl = rels[nm]
                t = pp.tile([P, max(rl.idx_width, 1)], i16, name="idx" + nm)
                nc.sync.dma_start(out=t[:], in_=idx_h[nm][:])
                idx_t[nm] = t
            wt = pp.tile([P, 14, H], f16, name="wt", tag="wt")
            nc.sync.dma_start(out=wt[:],
                              in_=wslab[:].rearrange("(s p) d -> p s d", p=P))
            pool_t = pp.tile([P, 3], f16, name="pool_t", tag="pool_t")
            nc.sync.dma_start(out=pool_t[:], in_=pool_in[:])
            bias_t = {}
            for k, nz in (("bias_p1", bias_nz["c1w_bl"]),
                          ("bias_a1", bias_nz["c1b_bl"]),
                          ("bias_p2", bias_nz["skipP_b"]),
                          ("bias_a2", bias_nz["skipA_b"])):
                if nz:
                    t = pp.tile([P, H], f32, name=k + "_t")
                    nc.sync.dma_start(out=t[:], in_=bias_in[k][:])
                    bias_t[k] = t

            xaT = pp.tile([P, A_PAD], f16, name="xaT", tag="xaT")
            nc.sync.dma_start_transpose(out=xaT[:], in_=xa_chunk[:])
            xpT = pp.tile([P, P_PAD], f16, name="xpT", tag="xpT")
            nc.sync.dma_start_transpose(out=xpT[:], in_=xp_chunk[:])

            # h1 tables: fp8 local shard -> Shared-output AllGather table
            # (fp16 local shard feeds the DMA transposes for L2 root terms)
            h1a_sh = dp.tile([NA_AG, H], f8, name="h1a_sh", tag="h1a_sh",
                             addr_space="Shared")
            h1p_sh = dp.tile([NP_AG, H], f8, name="h1p_sh", tag="h1p_sh",
                             addr_space="Shared")
            h1a_l8 = dp.tile([A_PAD, H], f8, name="h1a_l8", tag="h1a_l8")
            h1p_l8 = dp.tile([P_PAD, H], f8, name="h1p_l8", tag="h1p_l8")
            h1a_loc = dp.tile([A_PAD, H], f16, name="h1a_loc", tag="h1a_loc")
            h1p_loc = dp.tile([P_PAD, H], f16, name="h1p_loc", tag="h1p_loc")

            def conv(nm, table, elem, Wl, Wr, rootT, skipW, skipT, bias,
                     h_l8, h_loc, pool_ps, pool_last_col):
                rl = rels[nm]
                nslice = elem // P
                it = idx_t[nm]
                fetch = 2 * elem if rl.paired else elem
                mw = 2 if rl.paired else 1
                gathers = []
                for w in range(rl.n_win):
                    wc = int(rl.wcols[w])
                    cb = int(rl.col_base[w])
                    aggT = []
                    if wc:
                        msgs = mp.tile([P, MAXW, 256], f8, tag="msgs")
                        for (b, ioff, nidx, lcb) in rl.ops[w]:
                            b0 = b * rl.bank_rows
                            b1 = min(b0 + rl.bank_rows, rl.table_rows)
                            gathers.append(nc.gpsimd.dma_gather(
                                msgs[:, lcb:lcb + nidx // P, :fetch],
                                table[b0:b1, :],
                                it[:, ioff:ioff + nidx // 16],
                                nidx, nidx, fetch, single_packet=False))
                        mask_t = mk.tile([P, MAXM * WD], f8, tag="mask")
                        nc.sync.dma_start(
                            out=mask_t[:, :wc * mw * WD],
                            in_=mask_h[nm][:, cb * mw * WD:
                                           (cb + wc) * mw * WD])
                        aggs = []
                        for s in range(nslice):
                            aggs.append(psA.tile([P, WD], f32, tag="agg",
                                                 name="agg", space="PSUM"))
                        for i in range(wc):
                            for h in range(mw):
                                for s in range(nslice):
                                    nc.tensor.matmul(
                                        out=aggs[s][:],
                                        lhsT=msgs[:, i:i + 1,
                                                  (h * nslice + s) * P:
                                                  (h * nslice + s + 1) * P],
                                        rhs=mask_t[:, (mw * i + h) * WD:
                                                   (mw * i + h + 1) * WD],
                                        start=(i == 0 and h == 0),
                                        stop=(i == wc - 1 and h == mw - 1))
                        for s in range(nslice):
                            a = wk.tile([P, WD], f16, tag="aggT")
                            nc.scalar.copy(out=a[:], in_=aggs[s][:])
                            aggT.append(a)
                    for tl in range(min(WIN, rl.n_tiles - w * WIN)):
                        t = w * WIN + tl
                        lin = psL.tile([P, H], f32, tag="lin", space="PSUM")
                        first = True
                        if wc:
                            for s in range(nslice):
                                nc.tensor.matmul(
                                    out=lin[:],
                                    lhsT=aggT[s][:, tl * P:(tl + 1) * P],
                                    rhs=wt[:, Wl[s]:Wl[s] + 1, :],
                                    start=first, stop=False)
                                first = False
                        for s in range(nslice):
                            nc.tensor.matmul(
                                out=lin[:],
                                lhsT=rootT[s][:, t * P:(t + 1) * P],
                                rhs=wt[:, Wr[s]:Wr[s] + 1, :],
                                start=first,
                                stop=(skipW is None and s == nslice - 1))
                            first = False
                        if skipW is not None:
                            nc.tensor.matmul(
                                out=lin[:], lhsT=skipT[:, t * P:(t + 1) * P],
                                rhs=wt[:, skipW:skipW + 1, :],
                                start=False, stop=True)
                        h16 = wk.tile([P, H], f16, tag="h16")
                        if bias is None:
                            src = lin
                        else:
                            tmp = wk.tile([P, H], f32, tag="btmp")
                            nc.vector.tensor_add(out=tmp[:], in0=lin[:],
                                                 in1=bias[:])
                            src = tmp
                        nc.scalar.activation(out=h16[:], in_=src[:],
                                             func=relu_f)
                        if h_l8 is not None:
                            h8 = wk.tile([P, H], f8, tag="h8")
                            nc.scalar.activation(out=h8[:], in_=src[:],
                                                 func=relu_f)
                            nc.scalar.dma_start(
                                out=h_l8[t * P:(t + 1) * P, :], in_=h8[:])
                            nc.scalar.dma_start(
                                out=h_loc[t * P:(t + 1) * P, :], in_=h16[:])
                        if pool_ps is not None:
                            oc = pool_last_col if t == rl.n_tiles - 1 else 0
                            nc.tensor.matmul(
                                out=pool_ps[:], lhsT=pool_t[:, oc:oc + 1],
                                rhs=h16[:], start=(t == 0),
                                stop=(t == rl.n_tiles - 1),
                                skip_group_check=True)
                return gathers

            # -------- layer 1: authors (wb: src papers -> dst authors)
            conv("B1", xp_cmp, IN, [W["c1b_Wl"]], [W["c1b_Wr"]], [xaT],
                 None, None, bias_t.get("bias_a1"), h1a_l8, h1a_loc, None, 0)
            h1aT = []
            for s in range(2):
                t = pp.tile([P, A_PAD], f16, name=f"h1aT{s}", tag=f"h1aT{s}")
                nc.sync.dma_start_transpose(
                    out=t[:], in_=h1a_loc[:, s * P:(s + 1) * P])
                h1aT.append(t)
            nc.gpsimd.collective_compute(
                "AllGather", mybir.AluOpType.bypass, replica_groups=rg,
                ins=[h1a_l8.opt()], outs=[h1a_sh.opt()])

            # -------- layer 1: papers (writes: src authors -> dst papers)
            conv("W1", xa_cmp, IN, [W["c1w_Wl"]], [W["c1w_Wr"]], [xpT],
                 None, None, bias_t.get("bias_p1"), h1p_l8, h1p_loc, None, 0)
            h1pT = []
            for s in range(2):
                t = pp.tile([P, P_PAD], f16, name=f"h1pT{s}", tag=f"h1pT{s}")
                nc.sync.dma_start_transpose(
                    out=t[:], in_=h1p_loc[:, s * P:(s + 1) * P])
                h1pT.append(t)

            # -------- layer 2: papers (gathers h1a from shared table)
            pool_p = psP.tile([1, H], f32, name="pool_p", tag="pool_p",
                              space="PSUM")
            pool_a = psP.tile([1, H], f32, name="pool_a", tag="pool_a",
                              space="PSUM")
            gW2 = conv("W2", h1a_sh, H, [W["c2w_Wl0"], W["c2w_Wl1"]],
                       [W["c2w_Wr0"], W["c2w_Wr1"]], h1pT, W["skipP_W"], xpT,
                       bias_t.get("bias_p2"), None, None, pool_p, 1)

            # AG(h1p): L2-papers does not consume it; pin it behind the last
            # L2-papers gather so the scheduler cannot hoist its inline wait
            # into the middle of the gather stream
            ccP = nc.gpsimd.collective_compute(
                "AllGather", mybir.AluOpType.bypass, replica_groups=rg,
                ins=[h1p_l8.opt()], outs=[h1p_sh.opt()])
            _add_dep_helper(ccP.ins, gW2[-1].ins,
                            reason="keep AG(h1p) after L2-papers gathers")

            # -------- layer 2: authors
            conv("B2", h1p_sh, H, [W["c2b_Wl0"], W["c2b_Wl1"]],
                 [W["c2b_Wr0"], W["c2b_Wr1"]], h1aT, W["skipA_W"], xaT,
                 bias_t.get("bias_a2"), None, None, pool_a, 2)

            pool_sb = wk.tile([1, 2 * H], f32, tag="poolout")
            nc.vector.tensor_copy(out=pool_sb[:, 0:H], in_=pool_a[:])
            nc.vector.tensor_copy(out=pool_sb[:, H:2 * H], in_=pool_p[:])
            nc.sync.dma_start(out=out_pool[:], in_=pool_sb[:])

            if debug:
                nc.sync.dma_start(out=dbg_h1a[:], in_=h1a_loc[:])
                nc.sync.dma_start(out=dbg_h1p[:], in_=h1p_loc[:])

    nc.compile()
    return nc


def kernel(**inputs):
    debug = bool(int(os.environ.get("GNN_DEBUG", "0")))
    trace = bool(int(os.environ.get("GNN_TRACE", "0")))
    rels, in_maps, bias_nz = _prep(inputs)
    nc = _build(rels, bias_nz, debug=debug)
    res = bass_utils.run_bass_kernel_spmd(
        nc, in_maps, core_ids=list(range(C)), trace=trace)
    kernel.last_results = res

    pools = np.stack([res.results[c]["out_pool"] for c in range(C)])
    sum_a = pools[:, 0, :H].astype(np.float64).sum(axis=0)
    sum_p = pools[:, 0, H:].astype(np.float64).sum(axis=0)
    pooled = np.concatenate([sum_a / NA, sum_p / NP_])[None, :]
    W1 = np.asarray(inputs["cls_W1"], np.float64)
    b1 = np.asarray(inputs["cls_b1"], np.float64)
    W2 = np.asarray(inputs["cls_W2"], np.float64)
    b2 = np.asarray(inputs["cls_b2"], np.float64)
    h = np.maximum(pooled @ W1.T + b1, 0.0)
    out = h @ W2.T + b2
    return out.astype(np.float32)


# revision 42
# speedup vs baseline: 1.1640x; 1.0178x over previous
"""Trainium2 Bass kernel for the GSAT HeteroGNN problem (8 NeuronCores).

Self-contained: hardcodes shapes/sharding; only imports the concourse
toolchain.

Strategy (dst-node sharding, SPMD over 8 cores):
  - papers split into 8 canonical chunks of 12500 (padded 12544 = 98 tiles),
    authors 8 x 6250 (padded 6272 = 49 tiles).
  - edges live on their dst's owner core, laid out host-side into 128-slot
    columns per (4-tile window, src-bank); dma_gather (int16 idx) fetches
    fp8 source rows as [128, cols, feat].
  - segment-mean via host-precomputed fp8 masks streamed by DMA:
    mask[slot, dst_in_window] = 1/deg(dst); TensorE accumulates
    aggT[feat, 512] in PSUM per window (no on-device mask building).
  - L1 gathers read per-core COMPACT fp8 tables (only the <=32k rows this
    core references -> single bank, minimal column padding).
  - L1 outputs h1 are written twice: fp8 rows into a local chunk that a
    Shared-output AllGather assembles into a shared fp8 table (each rank
    contributes only its 1.6-3.2MB shard; the old Local-output AllGathers
    moved 77MB/core), and fp16 into a local chunk used for DMA transposes
    (L2 root terms).
  - L2 gathers read the shared fp8 h1 tables directly.
  - all DMA transposes are placed before any collective in program order
    (the scheduler serializes transposes with collectives).
  - global mean-pool via ones-column matmuls accumulating in PSUM; final
    2-layer MLP on host in fp64.
"""
import os
import sys

try:
    import concourse  # noqa: F401
except ImportError:  # toolchain location in the grading container
    sys.path.insert(0, "/opt/trn_rl_repo")

import numpy as np
import ml_dtypes
from concourse import bass, bacc, mybir, tile  # noqa: F401
from concourse import bass_utils
from concourse.bass import _add_dep_helper

dt = mybir.dt
F8 = ml_dtypes.float8_e4m3

# ---------------------------------------------------------------- constants
NA, NP_, E = 50000, 100000, 300000
IN, H, OUT = 128, 256, 16
C = 8                      # cores
P = 128                    # partitions
A_CAN, P_CAN = NA // C, NP_ // C              # 6250 / 12500
A_PAD = ((A_CAN + P - 1) // P) * P            # 6272
P_PAD = ((P_CAN + P - 1) // P) * P            # 12544
NA_AG, NP_AG = C * A_PAD, C * P_PAD           # 50176 / 100352
WIN = 4                    # dst tiles per PSUM window (512 dsts)
WD = WIN * P               # window width in dsts


class RelLayer:
    """Host-side layout for one (relation, layer): slot columns per
    (window, bank), uniform across cores (max-over-cores column counts),
    int16 gather indices and fp8 recip masks."""

    def __init__(self, row_of, dst_owner, dstl, n_dst_can, n_dst_pad,
                 recip_dst_local, table_rows, parity=None):
        # parity: per-core per-edge 0/1 within its pair-row (paired mode);
        # row_of then holds PAIR ids and slots dedupe by (window, pair)
        # row_of: [C] list of per-edge row ids (into this layer's table)
        # dst_owner/dstl: per-edge owner core and local dst id (global arrays
        # already split: row_of[c] aligned with dstl[c])
        self.n_tiles = n_dst_pad // P
        self.n_win = (self.n_tiles + WIN - 1) // WIN
        nb = (table_rows + 32767) // 32768
        self.n_banks = nb
        self.bank_rows = (table_rows + nb - 1) // nb
        self.table_rows = table_rows

        self.paired = parity is not None
        # per-core per-cell counts -> uniform ncols
        ncols = np.zeros((self.n_win, nb), np.int64)
        per_core = []
        for c in range(C):
            rows, dl = row_of[c], dstl[c]
            par = parity[c] if self.paired else np.zeros(len(rows), np.int64)
            w = dl // WD
            b = rows // self.bank_rows
            if self.paired:
                # one slot per distinct (window, pair-row)
                key = w * self.table_rows + rows
                uk, inv = np.unique(key, return_inverse=True)
                cnt = np.zeros((self.n_win, nb), np.int64)
                np.add.at(cnt, ((uk // self.table_rows),
                                (uk % self.table_rows) // self.bank_rows), 1)
            else:
                inv = None
                cnt = np.zeros((self.n_win, nb), np.int64)
                np.add.at(cnt, (w, b), 1)
            ncols = np.maximum(ncols, (cnt + P - 1) // P)
            per_core.append((rows, dl, w, b, par, inv))
        self.ncols = ncols

        # global column layout: window-major, bank-minor
        self.col_base = np.zeros(self.n_win + 1, np.int64)
        self.ops = []              # per window: list of (bank, ioff, nidx, lcb)
        ioff = 0
        col = 0
        for w in range(self.n_win):
            self.col_base[w] = col
            wops = []
            lcb = 0
            for b in range(nb):
                nco = int(ncols[w, b])
                if nco:
                    wops.append((b, ioff, nco * P, lcb))
                    ioff += nco * P // 16
                    lcb += nco
                    col += nco
            self.ops.append(wops)
        self.col_base[self.n_win] = col
        self.total_cols = col
        self.idx_width = ioff
        self.wcols = np.diff(self.col_base).astype(np.int64)
        self.max_wcols = int(self.wcols.max()) if col else 0
        self.total_idx = col * P

        # per-core idx + masks
        self.idx16 = np.zeros((C, P, max(self.idx_width, 1)), np.int16)
        self.masks = np.zeros((C, P, max(col, 1), WD), F8)  # repl. if paired
        cell_base = {}
        lcb_map = {}
        for w in range(self.n_win):
            for (b, io, nidx, lcb) in self.ops[w]:
                cell_base[(w, b)] = io
                lcb_map[(w, b)] = self.col_base[w] + lcb
        mw = 2 if self.paired else 1
        self.masks = np.zeros((C, P, max(col, 1) * mw, WD), F8)
        for c in range(C):
            rows, dl, w_e, b_e, par_e, inv = per_core[c]
            if self.paired:
                # slot per distinct (w, pair): compute slot ranks per cell
                key_sl = w_e * self.table_rows + rows
                uk = np.unique(key_sl)
                sw = uk // self.table_rows
                srow = uk % self.table_rows
                sb = srow // self.bank_rows
                cellkey = sw * nb + sb
                cellcnt = np.bincount(cellkey, minlength=self.n_win * nb)
                starts = np.zeros(self.n_win * nb + 1, np.int64)
                np.cumsum(cellcnt, out=starts[1:])
                jslot = np.arange(len(uk)) - starts[cellkey]
                # idx slab: one idx per slot
                flat = np.zeros(max(self.idx_width, 1) * 16, np.int16)
                iobase = np.array([cell_base.get((w, b), -1) * 16
                                   for w in range(self.n_win)
                                   for b in range(nb)]).reshape(self.n_win, nb)
                flat[iobase[sw, sb] + jslot] = (srow % self.bank_rows
                                                ).astype(np.int16)
                w16 = flat.reshape(-1, 16).T
                self.idx16[c] = np.tile(w16, (8, 1))
                # masks: edges land at their slot, plane = parity
                gcol = np.array([lcb_map.get((w, b), 0)
                                 for w in range(self.n_win)
                                 for b in range(nb)]).reshape(self.n_win, nb)
                slot_of_edge = np.searchsorted(uk, key_sl)
                je = jslot[slot_of_edge]
                cc = gcol[sw[slot_of_edge], sb[slot_of_edge]] + je // P
                pp = je % P
                off = dl - w_e * WD
                rec = recip_dst_local[c][dl].astype(np.float32)
                mbuf = np.zeros((P, max(col, 1) * mw, WD), np.float32)
                np.add.at(mbuf, (pp, cc * 2 + par_e, off), rec)
                self.masks[c] = mbuf.astype(F8)
                continue
            order = np.argsort(w_e * nb + b_e, kind="stable")
            rows_s, dl_s, w_s, b_s = rows[order], dl[order], w_e[order], b_e[order]
            rec_s = recip_dst_local[c][dl_s].astype(np.float32)
            rib_s = (rows_s % self.bank_rows).astype(np.int64)
            # rank within each (w, b) run
            key = w_s * nb + b_s
            # j = index within cell
            cellcnt = np.bincount(key, minlength=self.n_win * nb)
            starts = np.zeros(self.n_win * nb + 1, np.int64)
            np.cumsum(cellcnt, out=starts[1:])
            j = np.arange(len(key)) - starts[key]
            # idx slab (flat over ops)
            flat = np.zeros(max(self.idx_width, 1) * 16, np.int16)
            iobase = np.array([cell_base.get((w, b), -1) * 16
                               for w in range(self.n_win) for b in range(nb)]
                              ).reshape(self.n_win, nb)
            pos = iobase[w_s, b_s] + j
            flat[pos] = rib_s.astype(np.int16)
            w16 = flat.reshape(-1, 16).T       # [16, width]
            self.idx16[c] = np.tile(w16, (8, 1))
            # masks
            gcol = np.array([lcb_map.get((w, b), 0)
                             for w in range(self.n_win) for b in range(nb)]
                            ).reshape(self.n_win, nb)
            cc = gcol[w_s, b_s] + j // P
            pp = j % P
            off = dl_s - w_s * WD
            self.masks[c][pp, cc, off] = rec_s.astype(F8)


def _pair_srcs(srcs, dstls):
    """Per-core greedy pairing of gather sources by co-window occurrence.
    Returns per-core (pair_id per edge, parity per edge, pair row list)."""
    out = []
    for c in range(C):
        s, dl = srcs[c], dstls[c]
        w = dl // WD
        partner = {}
        for wi in range(int(w.max()) + 1):
            ss = np.unique(s[w == wi])
            free = [int(x) for x in ss if int(x) not in partner]
            for a, b in zip(free[0::2], free[1::2]):
                partner[a] = b
                partner[b] = a
        uniq = np.unique(s)
        left = [int(x) for x in uniq if int(x) not in partner]
        for a, b in zip(left[0::2], left[1::2]):
            partner[a] = b
            partner[b] = a
        if len(left) % 2:
            partner[left[-1]] = -1
        pair_rows = []
        pair_of, parity_of = {}, {}
        for x in uniq:
            x = int(x)
            if x in pair_of:
                continue
            p = partner[x]
            k = len(pair_rows)
            if p == -1:
                pair_rows.append((x, x))
                pair_of[x] = k
                parity_of[x] = 0
            else:
                pair_rows.append((x, p))
                pair_of[x] = k
                parity_of[x] = 0
                pair_of[p] = k
                parity_of[p] = 1
        ids = np.array([pair_of[int(x)] for x in s], np.int64)
        par = np.array([parity_of[int(x)] for x in s], np.int64)
        out.append((ids, par, pair_rows))
    return out


def _balance_perm(deg, n_nodes, can):
    """Permutation node -> new global id, dealing nodes into (core, window)
    cells so per-cell degree sums are balanced (pool is perm-invariant)."""
    import heapq
    n_win = ((can + P - 1) // P + WIN - 1) // WIN
    caps, base = [], []
    for c in range(C):
        for w in range(n_win):
            cap = min(WD, can - w * WD)
            caps.append(cap)
            base.append(c * can + w * WD)
    order = np.argsort(-deg, kind="stable")
    heap = [(0.0, i) for i in range(len(caps))]
    heapq.heapify(heap)
    fill = np.zeros(len(caps), np.int64)
    perm = np.empty(n_nodes, np.int64)
    for nd in order:
        while True:
            s, i = heapq.heappop(heap)
            if fill[i] < caps[i]:
                break
        perm[nd] = base[i] + fill[i]
        fill[i] += 1
        if fill[i] < caps[i]:
            heapq.heappush(heap, (s + deg[nd], i))
    return perm


def _prep(inputs):
    f = lambda k: np.asarray(inputs[k], np.float32)
    x_author, x_paper = f("x_author"), f("x_paper")
    ws, wd = (np.asarray(inputs["ei_writes_src"], np.int64),
              np.asarray(inputs["ei_writes_dst"], np.int64))
    bs, bd = (np.asarray(inputs["ei_wb_src"], np.int64),
              np.asarray(inputs["ei_wb_dst"], np.int64))

    # relabel nodes so per-(core, window) dst-degree sums are balanced
    pa_perm = _balance_perm(np.bincount(wd, minlength=NP_), NP_, P_CAN)
    au_perm = _balance_perm(np.bincount(bd, minlength=NA), NA, A_CAN)
    inv_pa = np.empty(NP_, np.int64)
    inv_pa[pa_perm] = np.arange(NP_)
    inv_au = np.empty(NA, np.int64)
    inv_au[au_perm] = np.arange(NA)
    x_paper = x_paper[inv_pa]
    x_author = x_author[inv_au]
    wd, bs = pa_perm[wd], pa_perm[bs]
    ws, bd = au_perm[ws], au_perm[bd]

    cnt_p = np.bincount(wd, minlength=NP_).astype(np.float32)
    cnt_a = np.bincount(bd, minlength=NA).astype(np.float32)
    recip_p = 1.0 / np.maximum(cnt_p, 1.0)
    recip_a = 1.0 / np.maximum(cnt_a, 1.0)

    # split edges by dst owner
    def split(src, dst, dst_can):
        srcs, dstls = [], []
        for c in range(C):
            m = (dst // dst_can) == c
            srcs.append(src[m])
            dstls.append((dst[m] % dst_can).astype(np.int64))
        return srcs, dstls

    w_src, w_dstl = split(ws, wd, P_CAN)     # writes: dst papers
    b_src, b_dstl = split(bs, bd, A_CAN)     # wb: dst authors

    # L1 compact PAIR tables: two co-window srcs per 256B fp8 row, so one
    # gather descriptor serves up to two edges
    pairsW = _pair_srcs(w_src, w_dstl)       # authors referenced per core
    pairsB = _pair_srcs(b_src, b_dstl)       # papers referenced per core
    rowsW = ((max(len(p[2]) for p in pairsW) + P - 1) // P) * P
    rowsB = ((max(len(p[2]) for p in pairsB) + P - 1) // P) * P
    assert rowsW <= 32768 and rowsB <= 32768
    xa_cmp = np.zeros((C, rowsW, 2 * IN), F8)
    xp_cmp = np.zeros((C, rowsB, 2 * IN), F8)
    for c in range(C):
        pr = np.array(pairsW[c][2], np.int64)
        xa_cmp[c, :len(pr), :IN] = x_author[pr[:, 0]].astype(F8)
        xa_cmp[c, :len(pr), IN:] = x_author[pr[:, 1]].astype(F8)
        pr = np.array(pairsB[c][2], np.int64)
        xp_cmp[c, :len(pr), :IN] = x_paper[pr[:, 0]].astype(F8)
        xp_cmp[c, :len(pr), IN:] = x_paper[pr[:, 1]].astype(F8)

    recip_p_loc = [recip_p[c * P_CAN:(c + 1) * P_CAN] for c in range(C)]
    recip_a_loc = [recip_a[c * A_CAN:(c + 1) * A_CAN] for c in range(C)]

    # AG row mapping for L2 tables
    agW = [(s // A_CAN) * A_PAD + (s % A_CAN) for s in w_src]
    agB = [(s // P_CAN) * P_PAD + (s % P_CAN) for s in b_src]
    rels = dict(
        W1=RelLayer([p[0] for p in pairsW], None, w_dstl, P_CAN, P_PAD,
                    recip_p_loc, rowsW, parity=[p[1] for p in pairsW]),
        B1=RelLayer([p[0] for p in pairsB], None, b_dstl, A_CAN, A_PAD,
                    recip_a_loc, rowsB, parity=[p[1] for p in pairsB]),
        W2=RelLayer(agW, None, w_dstl, P_CAN, P_PAD, recip_p_loc, NA_AG),
        B2=RelLayer(agB, None, b_dstl, A_CAN, A_PAD, recip_a_loc, NP_AG),
    )

    # fp16 local chunks (root/skip transposes)
    xa_chunk = np.zeros((C, A_PAD, IN), np.float16)
    xp_chunk = np.zeros((C, P_PAD, IN), np.float16)
    for c in range(C):
        xa_chunk[c, :A_CAN] = x_author[c * A_CAN:(c + 1) * A_CAN]
        xp_chunk[c, :P_CAN] = x_paper[c * P_CAN:(c + 1) * P_CAN]

    # weight slab: 14 x [128, 256] fp16 (transposed: [in, out])
    wT = lambda k: f(k).T.astype(np.float16)
    slabs = [wT("c1w_Wl"), wT("c1w_Wr"), wT("c1b_Wl"), wT("c1b_Wr")]
    for k in ("c2w_Wl", "c2w_Wr", "c2b_Wl", "c2b_Wr"):
        w2 = wT(k)
        slabs += [w2[:128], w2[128:]]
    slabs += [wT("skipA_W"), wT("skipP_W")]
    wslab = np.concatenate(slabs, axis=0)          # [14*128, 256]

    pool_ones = np.zeros((P, 3), np.float16)
    pool_ones[:, 0] = 1.0
    pool_ones[:P_CAN - (P_PAD // P - 1) * P, 1] = 1.0   # last paper tile mask
    pool_ones[:A_CAN - (A_PAD // P - 1) * P, 2] = 1.0   # last author tile mask

    bias_nz = {k: bool(np.any(f(k))) for k in
               ("c1w_bl", "c1b_bl", "skipA_b", "skipP_b")}
    bias_arr = {k: np.broadcast_to(f(k2), (P, H)).astype(np.float32).copy()
                for k, k2 in (("bias_p1", "c1w_bl"), ("bias_a1", "c1b_bl"),
                              ("bias_p2", "skipP_b"), ("bias_a2", "skipA_b"))}

    in_maps = []
    for c in range(C):
        m = dict(
            xa_cmp=xa_cmp[c], xp_cmp=xp_cmp[c],
            xa_chunk=xa_chunk[c], xp_chunk=xp_chunk[c],
            wslab=wslab, pool_ones=pool_ones,
        )
        for nm, rl in rels.items():
            m["idx_" + nm] = rl.idx16[c]
            m["mask_" + nm] = rl.masks[c].reshape(P, -1)
        for k, arr in bias_arr.items():
            m[k] = arr
        in_maps.append(m)
    return rels, in_maps, bias_nz


def _build(rels, bias_nz, debug=False):
    nc = bacc.Bacc("TRN2", target_bir_lowering=False, debug=False,
                   num_devices=C)
    f16, f32, i16, f8 = dt.float16, dt.float32, dt.int16, dt.float8e4
    ein = lambda n, s, d: nc.dram_tensor(n, s, d, kind="ExternalInput")

    xa_cmp = ein("xa_cmp", [rels["W1"].table_rows, 2 * IN], f8)
    xp_cmp = ein("xp_cmp", [rels["B1"].table_rows, 2 * IN], f8)
    xa_chunk = ein("xa_chunk", [A_PAD, IN], f16)
    xp_chunk = ein("xp_chunk", [P_PAD, IN], f16)
    wslab = ein("wslab", [14 * P, H], f16)
    pool_in = ein("pool_ones", [P, 3], f16)
    idx_h, mask_h = {}, {}
    for nm, rl in rels.items():
        mw = 2 if rl.paired else 1
        idx_h[nm] = ein("idx_" + nm, [P, max(rl.idx_width, 1)], i16)
        mask_h[nm] = ein("mask_" + nm, [P, max(rl.total_cols, 1) * mw * WD], f8)
    bias_in = {k: ein(k, [P, H], f32)
               for k in ("bias_p1", "bias_a1", "bias_p2", "bias_a2")}

    out_pool = nc.dram_tensor("out_pool", [1, 2 * H], f32,
                              kind="ExternalOutput")
    if debug:
        dbg_h1a = nc.dram_tensor("dbg_h1a", [A_PAD, H], f16,
                                 kind="ExternalOutput")
        dbg_h1p = nc.dram_tensor("dbg_h1p", [P_PAD, H], f16,
                                 kind="ExternalOutput")

    W = {k: i for i, k in enumerate(
        ["c1w_Wl", "c1w_Wr", "c1b_Wl", "c1b_Wr",
         "c2w_Wl0", "c2w_Wl1", "c2w_Wr0", "c2w_Wr1",
         "c2b_Wl0", "c2b_Wl1", "c2b_Wr0", "c2b_Wr1",
         "skipA_W", "skipP_W"])}
    relu_f = mybir.ActivationFunctionType.Relu
    rg = [list(range(C))]
    MAXW = max(rl.max_wcols for rl in rels.values())
    MAXM = max(rl.max_wcols * (2 if rl.paired else 1) for rl in rels.values())

    with tile.TileContext(nc) as tc:
        with tc.tile_pool(name="persist", bufs=1) as pp, \
             tc.tile_pool(name="dram", bufs=1, space="DRAM") as dp, \
             tc.tile_pool(name="work", bufs=3) as wk, \
             tc.tile_pool(name="msgs", bufs=3) as mp, \
             tc.tile_pool(name="maskp", bufs=2) as mk, \
             tc.tile_pool(name="psA", bufs=4, space="PSUM") as psA, \
             tc.tile_pool(name="psL", bufs=2, space="PSUM") as psL, \
             tc.tile_pool(name="psP", bufs=1, space="PSUM") as psP:

            # ---------------- persistent loads (idx first: gathers need it)
            idx_t = {}
            for nm in ("B1", "W1", "W2", "B2"):
                rl = rels[nm]
                t = pp.tile([P, max(rl.idx_width, 1)], i16, name="idx" + nm)
                nc.sync.dma_start(out=t[:], in_=idx_h[nm][:])
                idx_t[nm] = t
            wt = pp.tile([P, 14, H], f16, name="wt", tag="wt")
            nc.sync.dma_start(out=wt[:],
                              in_=wslab[:].rearrange("(s p) d -> p s d", p=P))
            pool_t = pp.tile([P, 3], f16, name="pool_t", tag="pool_t")
            nc.sync.dma_start(out=pool_t[:], in_=pool_in[:])
            bias_t = {}
            for k, nz in (("bias_p1", bias_nz["c1w_bl"]),
                          ("bias_a1", bias_nz["c1b_bl"]),
                          ("bias_p2", bias_nz["skipP_b"]),
                          ("bias_a2", bias_nz["skipA_b"])):
                if nz:
                    t = pp.tile([P, H], f32, name=k + "_t")
                    nc.sync.dma_start(out=t[:], in_=bias_in[k][:])
                    bias_t[k] = t

            xaT = pp.tile([P, A_PAD], f16, name="xaT", tag="xaT")
            nc.sync.dma_start_transpose(out=xaT[:], in_=xa_chunk[:])
            xpT = pp.tile([P, P_PAD], f16, name="xpT", tag="xpT")
            nc.sync.dma_start_transpose(out=xpT[:], in_=xp_chunk[:])

            # h1 tables: fp8 local shard -> Shared-output AllGather table
            # (fp16 local shard feeds the DMA transposes for L2 root terms)
            h1a_sh = dp.tile([NA_AG, H], f8, name="h1a_sh", tag="h1a_sh",
                             addr_space="Shared")
            h1p_sh = dp.tile([NP_AG, H], f8, name="h1p_sh", tag="h1p_sh",
                             addr_space="Shared")
            h1a_l8 = dp.tile([A_PAD, H], f8, name="h1a_l8", tag="h1a_l8")
            h1p_l8 = dp.tile([P_PAD, H], f8, name="h1p_l8", tag="h1p_l8")
            h1a_loc = dp.tile([A_PAD, H], f16, name="h1a_loc", tag="h1a_loc")
            h1p_loc = dp.tile([P_PAD, H], f16, name="h1p_loc", tag="h1p_loc")

            def conv(nm, table, elem, Wl, Wr, rootT, skipW, skipT, bias,
                     h_l8, h_loc, pool_ps, pool_last_col):
                rl = rels[nm]
                nslice = elem // P
                it = idx_t[nm]
                fetch = 2 * elem if rl.paired else elem
                mw = 2 if rl.paired else 1
                gathers = []
                for w in range(rl.n_win):
                    wc = int(rl.wcols[w])
                    cb = int(rl.col_base[w])
                    aggT = []
                    if wc:
                        msgs = mp.tile([P, MAXW, 256], f8, tag="msgs")
                        for (b, ioff, nidx, lcb) in rl.ops[w]:
                            b0 = b * rl.bank_rows
                            b1 = min(b0 + rl.bank_rows, rl.table_rows)
                            gathers.append(nc.gpsimd.dma_gather(
                                msgs[:, lcb:lcb + nidx // P, :fetch],
                                table[b0:b1, :],
                                it[:, ioff:ioff + nidx // 16],
                                nidx, nidx, fetch, single_packet=False))
                        mask_t = mk.tile([P, MAXM * WD], f8, tag="mask")
                        nc.sync.dma_start(
                            out=mask_t[:, :wc * mw * WD],
                            in_=mask_h[nm][:, cb * mw * WD:
                                           (cb + wc) * mw * WD])
                        aggs = []
                        for s in range(nslice):
                            aggs.append(psA.tile([P, WD], f32, tag="agg",
                                                 name="agg", space="PSUM"))
                        for i in range(wc):
                            for h in range(mw):
                                for s in range(nslice):
                                    nc.tensor.matmul(
                                        out=aggs[s][:],
                                        lhsT=msgs[:, i:i + 1,
                                                  (h * nslice + s) * P:
                                                  (h * nslice + s + 1) * P],
                                        rhs=mask_t[:, (mw * i + h) * WD:
                                                   (mw * i + h + 1) * WD],
                                        start=(i == 0 and h == 0),
                                        stop=(i == wc - 1 and h == mw - 1))
                        for s in range(nslice):
                            a = wk.tile([P, WD], f16, tag="aggT")
                            nc.scalar.copy(out=a[:], in_=aggs[s][:])
                            aggT.append(a)
                    for tl in range(min(WIN, rl.n_tiles - w * WIN)):
                        t = w * WIN + tl
                        lin = psL.tile([P, H], f32, tag="lin", space="PSUM")
                        first = True
                        if wc:
                            for s in range(nslice):
                                nc.tensor.matmul(
                                    out=lin[:],
                                    lhsT=aggT[s][:, tl * P:(tl + 1) * P],
                                    rhs=wt[:, Wl[s]:Wl[s] + 1, :],
                                    start=first, stop=False)
                                first = False
                        for s in range(nslice):
                            nc.tensor.matmul(
                                out=lin[:],
                                lhsT=rootT[s][:, t * P:(t + 1) * P],
                                rhs=wt[:, Wr[s]:Wr[s] + 1, :],
                                start=first,
                                stop=(skipW is None and s == nslice - 1))
                            first = False
                        if skipW is not None:
                            nc.tensor.matmul(
                                out=lin[:], lhsT=skipT[:, t * P:(t + 1) * P],
                                rhs=wt[:, skipW:skipW + 1, :],
                                start=False, stop=True)
                        h16 = wk.tile([P, H], f16, tag="h16")
                        if bias is None:
                            src = lin
                        else:
                            tmp = wk.tile([P, H], f32, tag="btmp")
                            nc.vector.tensor_add(out=tmp[:], in0=lin[:],
                                                 in1=bias[:])
                            src = tmp
                        nc.scalar.activation(out=h16[:], in_=src[:],
                                             func=relu_f)
                        if h_l8 is not None:
                            h8 = wk.tile([P, H], f8, tag="h8")
                            nc.scalar.activation(out=h8[:], in_=src[:],
                                                 func=relu_f)
                            nc.sync.dma_start(
                                out=h_l8[t * P:(t + 1) * P, :], in_=h8[:])
                            nc.sync.dma_start(
                                out=h_loc[t * P:(t + 1) * P, :], in_=h16[:])
                        if pool_ps is not None:
                            oc = pool_last_col if t == rl.n_tiles - 1 else 0
                            nc.tensor.matmul(
                                out=pool_ps[:], lhsT=pool_t[:, oc:oc + 1],
                                rhs=h16[:], start=(t == 0),
                                stop=(t == rl.n_tiles - 1),
                                skip_group_check=True)
                return gathers

            # -------- layer 1: authors (wb: src papers -> dst authors)
            conv("B1", xp_cmp, IN, [W["c1b_Wl"]], [W["c1b_Wr"]], [xaT],
                 None, None, bias_t.get("bias_a1"), h1a_l8, h1a_loc, None, 0)
            h1aT = []
            for s in range(2):
                t = pp.tile([P, A_PAD], f16, name=f"h1aT{s}", tag=f"h1aT{s}")
                nc.sync.dma_start_transpose(
                    out=t[:], in_=h1a_loc[:, s * P:(s + 1) * P])
                h1aT.append(t)
            nc.gpsimd.collective_compute(
                "AllGather", mybir.AluOpType.bypass, replica_groups=rg,
                ins=[h1a_l8.opt()], outs=[h1a_sh.opt()])

            # -------- layer 1: papers (writes: src authors -> dst papers)
            conv("W1", xa_cmp, IN, [W["c1w_Wl"]], [W["c1w_Wr"]], [xpT],
                 None, None, bias_t.get("bias_p1"), h1p_l8, h1p_loc, None, 0)
            h1pT = []
            for s in range(2):
                t = pp.tile([P, P_PAD], f16, name=f"h1pT{s}", tag=f"h1pT{s}")
                nc.sync.dma_start_transpose(
                    out=t[:], in_=h1p_loc[:, s * P:(s + 1) * P])
                h1pT.append(t)

            # -------- layer 2: papers (gathers h1a from shared table)
            pool_p = psP.tile([1, H], f32, name="pool_p", tag="pool_p",
                              space="PSUM")
            pool_a = psP.tile([1, H], f32, name="pool_a", tag="pool_a",
                              space="PSUM")
            gW2 = conv("W2", h1a_sh, H, [W["c2w_Wl0"], W["c2w_Wl1"]],
                       [W["c2w_Wr0"], W["c2w_Wr1"]], h1pT, W["skipP_W"], xpT,
                       bias_t.get("bias_p2"), None, None, pool_p, 1)

            # AG(h1p): L2-papers does not consume it; pin it behind the last
            # L2-papers gather so the scheduler cannot hoist its inline wait
            # into the middle of the gather stream
            ccP = nc.gpsimd.collective_compute(
                "AllGather", mybir.AluOpType.bypass, replica_groups=rg,
                ins=[h1p_l8.opt()], outs=[h1p_sh.opt()])
            _add_dep_helper(ccP.ins, gW2[-1].ins,
                            reason="keep AG(h1p) after L2-papers gathers")

            # -------- layer 2: authors
            conv("B2", h1p_sh, H, [W["c2b_Wl0"], W["c2b_Wl1"]],
                 [W["c2b_Wr0"], W["c2b_Wr1"]], h1aT, W["skipA_W"], xaT,
                 bias_t.get("bias_a2"), None, None, pool_a, 2)

            pool_sb = wk.tile([1, 2 * H], f32, tag="poolout")
            nc.vector.tensor_copy(out=pool_sb[:, 0:H], in_=pool_a[:])
            nc.vector.tensor_copy(out=pool_sb[:, H:2 * H], in_=pool_p[:])
            nc.sync.dma_start(out=out_pool[:], in_=pool_sb[:])

            if debug:
                nc.sync.dma_start(out=dbg_h1a[:], in_=h1a_loc[:])
                nc.sync.dma_start(out=dbg_h1p[:], in_=h1p_loc[:])

    nc.compile()
    return nc


def kernel(**inputs):
    debug = bool(int(os.environ.get("GNN_DEBUG", "0")))
    trace = bool(int(os.environ.get("GNN_TRACE", "0")))
    rels, in_maps, bias_nz = _prep(inputs)
    nc = _build(rels, bias_nz, debug=debug)
    res = bass_utils.run_bass_kernel_spmd(
        nc, in_maps, core_ids=list(range(C)), trace=trace)
    kernel.last_results = res

    pools = np.stack([res.results[c]["out_pool"] for c in range(C)])
    sum_a = pools[:, 0, :H].astype(np.float64).sum(axis=0)
    sum_p = pools[:, 0, H:].astype(np.float64).sum(axis=0)
    pooled = np.concatenate([sum_a / NA, sum_p / NP_])[None, :]
    W1 = np.asarray(inputs["cls_W1"], np.float64)
    b1 = np.asarray(inputs["cls_b1"], np.float64)
    W2 = np.asarray(inputs["cls_W2"], np.float64)
    b2 = np.asarray(inputs["cls_b2"], np.float64)
    h = np.maximum(pooled @ W1.T + b1, 0.0)
    out = h @ W2.T + b2
    return out.astype(np.float32)
